# revision 2
# baseline (speedup 1.0000x reference)
"""Multi-head attention (B=2, S=2048, D=1024, H=16) on 8 Trainium2 cores.

Sharding: core = b*4 + g  ->  batch b (data parallel), head-group g of 4
heads (tensor parallel).  Each core computes a partial out^T = Wo_g^T @ Z_g
for its batch; the host sums the 4 partials per batch (the "all-reduce"),
transposes back and adds the (folded) output bias.

All activations flow feature-major on device (x^T, Q^T, K^T, scores^T) so
no on-device transposes are needed.  Matmuls run in bf16 with fp32 PSUM
accumulation.  Softmax skips the row-max pass (scores are bounded), gets
its denominator from a ones-column appended to V, and defers normalization
to after the attention*V matmul.  The reciprocal of the denominator is
broadcast across partitions with a rank-1 PE matmul (K=1) instead of a
DRAM round trip, and its instructions are emitted a few blocks late so
the in-order PE queue never stalls waiting on it.  Output partials are
written back in bf16 (the host accumulates in fp32).
"""

import numpy as np
import ml_dtypes

B, S, D, H = 2, 2048, 1024, 16
DK = D // H                  # 64
SCALE = 1.0 / np.sqrt(D)
NCORES = 8
GROUPS = 4                   # head-groups (tensor parallel)
HG = H // GROUPS             # 4 heads per group
DG = D // GROUPS             # 256 head dims per group
P = 128
KO = D // P                  # 8 contraction chunks for the projections
MO = DG // P                 # 2 row-chunks of Q^T/K^T (= head pairs)
NQ = 512                     # q tile width
QT = S // NQ                 # 4
ST = S // P                  # 16 key blocks / s chunks
BF16 = ml_dtypes.bfloat16

_cache = {}


def _classify_mask(mask):
    """Block structure of mask^T ([k, q] layout, P x NQ blocks).

    Returns (cls, qoff, mixed_idx, mixed_tiles, use_affine):
      cls[kt][qt]  : 0 all-masked, 1 all-kept, 2 mixed
      qoff[kt][qt] : leading all-masked columns (trim), 0 unless tril
      mixed_idx    : {(kt, qt): index into mixed_tiles}
      mixed_tiles  : np [n, P, NQ] bf16 0/1 tiles (empty when use_affine)
    """
    tril = np.tril(np.ones((S, S), dtype=mask.dtype))
    use_affine = bool(np.array_equal(mask, tril))
    cls = [[1] * QT for _ in range(ST)]
    qoff = [[0] * QT for _ in range(ST)]
    mixed_idx = {}
    tiles = []
    if use_affine:
        for kt in range(ST):
            k0 = kt * P
            for qt in range(QT):
                q0 = qt * NQ
                if k0 - q0 >= NQ:
                    cls[kt][qt] = 0
                elif k0 + P - 1 > q0:
                    cls[kt][qt] = 2
                    qoff[kt][qt] = min(max(k0 - q0, 0), NQ - P)
                # else: fully kept
    else:
        keepT = (mask != 0).T        # [k, q]
        for kt in range(ST):
            for qt in range(QT):
                blk = keepT[kt * P:(kt + 1) * P, qt * NQ:(qt + 1) * NQ]
                if not blk.any():
                    cls[kt][qt] = 0
                elif blk.all():
                    cls[kt][qt] = 1
                else:
                    cls[kt][qt] = 2
                    mixed_idx[(kt, qt)] = len(tiles)
                    tiles.append(blk.astype(BF16))
    mixed_tiles = (np.stack(tiles) if tiles else
                   np.zeros((0, P, NQ), dtype=BF16))
    return cls, qoff, mixed_idx, mixed_tiles, use_affine


def _build_program(cls, qoff, mixed_idx, n_mixed, use_affine):
    from contextlib import ExitStack
    import concourse.bass as bass
    import concourse.tile as tile
    import concourse.mybir as mybir
    from concourse import bacc
    from concourse.bass import ds, ts

    f32 = mybir.dt.float32
    bf16 = mybir.dt.bfloat16
    Exp = mybir.ActivationFunctionType.Exp

    nc = bacc.Bacc(None, target_bir_lowering=False, name="mha_tp")

    xT = nc.dram_tensor("xT", [D, S], bf16, kind="ExternalInput")
    wq = nc.dram_tensor("wq", [D, DG], bf16, kind="ExternalInput")
    wk = nc.dram_tensor("wk", [D, DG], bf16, kind="ExternalInput")
    wv = nc.dram_tensor("wv", [D, DG], bf16, kind="ExternalInput")
    wo = nc.dram_tensor("wo", [DG, D], bf16, kind="ExternalInput")
    bqk = nc.dram_tensor("bqk", [2, DG], f32, kind="ExternalInput")
    mm = (nc.dram_tensor("mmask", [n_mixed, P, NQ], bf16, kind="ExternalInput")
          if n_mixed else None)
    outT = nc.dram_tensor("outT", [D, S], bf16, kind="ExternalOutput")

    xTv = xT.ap().rearrange("(ko p) s -> p ko s", p=P)
    wqv = wq.ap().rearrange("(ko p) m -> p ko m", p=P)
    wkv = wk.ap().rearrange("(ko p) m -> p ko m", p=P)
    wvv = wv.ap().rearrange("(ko p) m -> p ko m", p=P)
    wov = wo.ap().rearrange("(zo p) n -> p zo n", p=P)
    bqkv = bqk.ap().rearrange("t (mo p) -> p t mo", p=P)
    outv = outT.ap().rearrange("(mo p) s -> p mo s", p=P)

    with tile.TileContext(nc) as tc, ExitStack() as ctx:
        const = ctx.enter_context(tc.tile_pool(name="const", bufs=1))

        # DMA order matters: the first QKV matmul needs wq + the first x
        # column slab; interleave them in small pieces so the lead-in is
        # paced by queue parallelism, not one big serial transfer.
        bias_sb = const.tile([P, 2, 2], f32)
        nc.sync.dma_start(bias_sb[:], bqkv)
        wq_sb = const.tile([P, KO, DG], bf16)
        x_sb = const.tile([P, KO, S], bf16)
        for ko in range(KO):
            nc.sync.dma_start(wq_sb[:, ko, :], wqv[:, ko, :])
            nc.sync.dma_start(x_sb[:, ko, ts(0, NQ)], xTv[:, ko, ts(0, NQ)])
        wk_sb = const.tile([P, KO, DG], bf16)
        for ko in range(KO):
            nc.sync.dma_start(wk_sb[:, ko, :], wkv[:, ko, :])
        wv_sb = const.tile([P, KO, DG], bf16)
        for ko in range(KO):
            nc.sync.dma_start(wv_sb[:, ko, :], wvv[:, ko, :])
        for ko in range(KO):
            nc.sync.dma_start(x_sb[:, ko, ts(1, NQ)], xTv[:, ko, ts(1, NQ)])
        mask_sb = None
        if n_mixed:
            mask_sb = const.tile([P, n_mixed, NQ], bf16)
            for i in range(n_mixed):
                nc.sync.dma_start(mask_sb[:, i, :], mm.ap()[i])
        wo_sb = const.tile([P, MO, D], bf16)
        for zo in range(MO):
            nc.sync.dma_start(wo_sb[:, zo, :], wov[:, zo, :])
        for qt in range(2, QT):
            for ko in range(KO):
                nc.sync.dma_start(x_sb[:, ko, ts(qt, NQ)],
                                  xTv[:, ko, ts(qt, NQ)])

        qT_sb = const.tile([P, MO, S], bf16)
        kT_sb = const.tile([P, MO, S], bf16)
        v_sb = const.tile([P, ST, HG, DK + 1], bf16)
        zT_sb = const.tile([P, MO, S], bf16)
        nc.gpsimd.memset(v_sb[:, :, :, DK:DK + 1], 1.0)
        # all-ones row used by the rank-1 reciprocal-broadcast matmul;
        # kept full-height so partition-64 slices exist.
        ones_sb = const.tile([P, DK], f32)
        nc.gpsimd.memset(ones_sb[:], 1.0)
        warm_sb = const.tile([P, DK], bf16)
        nc.gpsimd.memset(warm_sb[:], 0.0)

        with (
            tc.tile_pool(name="pqkv", bufs=2, space="PSUM") as pqkv,
            tc.tile_pool(name="ps_at", bufs=2, space="PSUM") as ps_at,
            tc.tile_pool(name="pz", bufs=1, space="PSUM") as pz,
            tc.tile_pool(name="work", bufs=8) as work,
            tc.tile_pool(name="rwork", bufs=3) as rwork,
        ):
            # keep the PE busy during the DMA lead-in so the HAM clock
            # gate is already released when the first real matmul issues
            zp_warm = pz.tile([P, 2, NQ], f32, tag="z")
            for _ in range(28):
                nc.tensor.matmul(zp_warm[0:DK, 0, 0:DK], warm_sb[:, :],
                                 warm_sb[:, :], start=True, stop=True)

            def av(zp, mo, prev, last):
                kt, pT, off, first = prev
                ret = None
                for h in (0, 1):
                    ret = nc.tensor.matmul(
                        zp[0:DK + 1, h, off:],
                        v_sb[:, kt, 2 * mo + h, :],
                        pT[:, h, off:],
                        start=first, stop=last)
                return ret

            def outproj_chunk(qt, mo8, split=1):
                nw = NQ // split
                for s in range(split):
                    o_ps = pqkv.tile([P, NQ], f32, tag="ps", name=f"o{mo8}")
                    for zo in range(MO):
                        nc.tensor.matmul(
                            o_ps[:, 0:nw], wo_sb[:, zo, ts(mo8, P)],
                            zT_sb[:, zo, ds(qt * NQ + s * nw, nw)],
                            start=(zo == 0), stop=(zo == MO - 1))
                    o_sb = work.tile([P, NQ], bf16, tag="osb")
                    if (mo8 + s) % 2 == 0:
                        nc.vector.tensor_copy(o_sb[:, 0:nw], o_ps[:, 0:nw])
                    else:
                        nc.scalar.copy(o_sb[:, 0:nw], o_ps[:, 0:nw])
                    nc.sync.dma_start(
                        outv[:, mo8, ds(qt * NQ + s * nw, nw)], o_sb[:, 0:nw])

            def qkv_slab(qt, mid_hook=None):
                for t, (w_sb, dst) in enumerate(((wq_sb, qT_sb),
                                                 (wk_sb, kT_sb))):
                    for mo in range(MO):
                        ps = pqkv.tile([P, NQ], f32, tag="ps")
                        for ko in range(KO):
                            nc.tensor.matmul(
                                ps, w_sb[:, ko, ts(mo, P)],
                                x_sb[:, ko, ts(qt, NQ)],
                                start=(ko == 0), stop=(ko == KO - 1))
                        nc.vector.tensor_scalar_add(
                            dst[:, mo, ts(qt, NQ)], ps,
                            bias_sb[:, t, mo:mo + 1])
                    if t == 0 and mid_hook is not None:
                        mid_hook()
                for so in range(HG * qt, HG * (qt + 1)):
                    ps = pqkv.tile([P, NQ], f32, tag="ps")
                    for ko in range(KO):
                        nc.tensor.matmul(
                            ps[:, :DG], x_sb[:, ko, ts(so, P)],
                            wv_sb[:, ko, :],
                            start=(ko == 0), stop=(ko == KO - 1))
                    nc.vector.tensor_copy(
                        v_sb[:, so, :, 0:DK],
                        ps[:, :DG].rearrange("p (h d) -> p h d", h=HG))

            # --- deferred softmax normalization ---------------------------
            # norm_a (emitted inline, right after the slab's last AV):
            #   reciprocal of the denominator row + copy of raw z out of
            #   PSUM (frees the bank).  norm_b (emitted a little later, so
            #   the PE queue has covering work): rank-1 broadcast matmul of
            #   the reciprocal + the normalizing multiplies.
            pending_nb = [None]

            def norm_a(mo, qt, zp, h1_on_act):
                r_row = rwork.tile([DK + 1, 2, NQ], f32, tag="rrow")
                nc.vector.reciprocal(r_row[DK:DK + 1, :, :],
                                     zp[DK:DK + 1, :, :])
                zraw = rwork.tile([DK, 2, NQ], f32, tag="zraw")
                if h1_on_act:
                    nc.vector.tensor_copy(zraw[:, 0, :], zp[0:DK, 0, :])
                    nc.scalar.copy(zraw[:, 1, :], zp[0:DK, 1, :])
                else:
                    nc.vector.tensor_copy(zraw[:], zp[0:DK, :, :])
                pending_nb[0] = (mo, qt, zraw, r_row)

            def norm_b():
                if pending_nb[0] is None:
                    return
                mo, qt, zraw, r_row = pending_nb[0]
                pending_nb[0] = None
                rb = ps_at.tile([P, 2, NQ], f32, tag="s", name="rb")
                for h in (0, 1):
                    nc.tensor.matmul(
                        rb[0:DK, h, :], ones_sb[DK:DK + 1, :],
                        r_row[DK:DK + 1, h, :], start=True, stop=True)
                nc.vector.tensor_mul(
                    zT_sb[0:DK, mo, ts(qt, NQ)], zraw[:, 0, :],
                    rb[0:DK, 0, :])
                zn_tmp = rwork.tile([DK, NQ], bf16, tag="zt")
                nc.vector.tensor_mul(zn_tmp[:], zraw[:, 1, :],
                                     rb[0:DK, 1, :])
                nc.sync.dma_start(zT_sb[DK:P, mo, ts(qt, NQ)], zn_tmp[:])

            if not use_affine:
                # a general mask may attend beyond block qt, so all K/V
                # slabs must exist before any attention starts
                for qt in range(QT):
                    qkv_slab(qt)

            proc = list(range(QT))
            emitted = 0
            prev_qt = None
            for qt in proc:
                q0 = qt * NQ
                if use_affine:
                    # attention(qt) only needs k blocks <= qt, so emit QKV
                    # slabs lazily just ahead of it; the previous slab's
                    # last norm_b lands between the Q and K projections.
                    while emitted <= qt:
                        qkv_slab(emitted,
                                 mid_hook=norm_b if emitted == qt else None)
                        emitted += 1
                else:
                    norm_b()

                # -- attention over k blocks of this slab -----------------
                for mo in range(MO):
                    kts = [kt for kt in range(ST) if cls[kt][qt] != 0]
                    if not kts:
                        nc.vector.memset(zT_sb[:, mo, ts(qt, NQ)], 0.0)
                        continue
                    # out-projection of the previously processed q slab
                    # rides along inside this kt stream (mo==1); hold back
                    # 3 chunks of the penultimate slab as covering PE work
                    # for the final normalization chain
                    outq = []
                    if mo == 1 and prev_qt is not None:
                        nch = 8 if qt != QT - 1 else 5
                        outq = [(prev_qt, j) for j in range(nch)]
                    per = -(-len(outq) // len(kts)) if outq else 0
                    zp = pz.tile([P, 2, NQ], f32, tag="z")
                    prev = None
                    for i, kt in enumerate(kts):
                        k0 = kt * P
                        off = qoff[kt][qt]
                        w = NQ - off
                        pT = work.tile([P, 2, NQ], bf16, tag="pT")
                        s_ps = ps_at.tile([P, 2, NQ], f32, tag="s")
                        for h in (0, 1):
                            hp = slice(h * DK, (h + 1) * DK)
                            nc.tensor.matmul(
                                s_ps[:, h, off:],
                                kT_sb[hp, mo, ts(kt, P)],
                                qT_sb[hp, mo, ds(q0 + off, w)],
                                start=True, stop=True,
                                tile_position=(h * DK, 0))
                        nc.scalar.activation(
                            pT[:, :, off:], s_ps[:, :, off:], Exp)
                        if cls[kt][qt] == 2:
                            if use_affine:
                                nc.gpsimd.affine_select(
                                    out=pT[:, :, off:],
                                    in_=pT[:, :, off:],
                                    compare_op=mybir.AluOpType.is_ge,
                                    fill=0.0,
                                    base=q0 + off - k0,
                                    channel_multiplier=-1,
                                    pattern=[[0, 2], [1, w]])
                            else:
                                nc.vector.tensor_mul(
                                    pT[:, :, off:], pT[:, :, off:],
                                    mask_sb[:, mixed_idx[(kt, qt)], None,
                                            off:].to_broadcast((P, 2, w)))
                        if prev is not None:
                            av(zp, mo, prev, last=False)
                        prev = (kt, pT, off, i == 0)
                        if mo == 1 and i == 1:
                            norm_b()   # this slab's mo=0 normalization
                        for qtd, j in outq[per * i: per * (i + 1)]:
                            outproj_chunk(qtd, j)
                    for qtd, j in outq[per * len(kts):]:
                        outproj_chunk(qtd, j)
                    av(zp, mo, prev, last=True)
                    norm_a(mo, qt, zp,
                           h1_on_act=(qt == proc[-1] and mo == MO - 1))
                prev_qt = qt

            # -- tail: held-back chunks cover the final norm chain --------
            if QT >= 2:
                outproj_chunk(QT - 2, 5)
                outproj_chunk(QT - 2, 6)
                norm_b()
                outproj_chunk(QT - 2, 7)
            else:
                norm_b()
            for mo8 in range(D // P):
                outproj_chunk(prev_qt, mo8, split=2)

    return nc


def _get_program(mask):
    cls, qoff, mixed_idx, mixed_tiles, use_affine = _classify_mask(mask)
    key = (use_affine,
           tuple(tuple(r) for r in cls),
           tuple(tuple(r) for r in qoff))
    if key not in _cache:
        nc = _build_program(cls, qoff, mixed_idx, len(mixed_tiles), use_affine)
        nc.compile()
        _cache[key] = nc
    return _cache[key], mixed_tiles


def _prep_in_maps(x, mask, Wq, bq, Wk, bk, Wv, bv, Wo, bo, mixed_tiles):
    xT = [np.ascontiguousarray(x[b].T).astype(BF16) for b in range(B)]
    in_maps = []
    for core in range(NCORES):
        b, g = divmod(core, GROUPS)
        c0, c1 = g * DG, (g + 1) * DG
        im = {
            "xT": xT[b],
            "wq": np.ascontiguousarray(Wq[:, c0:c1] * SCALE).astype(BF16),
            "wk": np.ascontiguousarray(Wk[:, c0:c1]).astype(BF16),
            "wv": np.ascontiguousarray(Wv[:, c0:c1]).astype(BF16),
            "wo": np.ascontiguousarray(Wo[c0:c1, :]).astype(BF16),
            "bqk": np.ascontiguousarray(
                np.stack([bq[c0:c1] * SCALE, bk[c0:c1]])).astype(np.float32),
        }
        if len(mixed_tiles):
            im["mmask"] = mixed_tiles
        in_maps.append(im)
    return in_maps


def _unshard(results, Wo, bv, bo):
    bo_eff = (bo.astype(np.float32)
              + bv.astype(np.float32) @ Wo.astype(np.float32))
    out = np.empty((B, S, D), np.float32)
    for b in range(B):
        acc = results[b * GROUPS]["outT"].astype(np.float32)
        for g in range(1, GROUPS):
            acc += results[b * GROUPS + g]["outT"]
        out[b] = acc.T + bo_eff
    return out


def kernel(trace=False, **inputs):
    from concourse import bass_utils

    args = {k: np.asarray(v) for k, v in inputs.items()}
    x, mask = args["x"], args["mask"]
    Wq, bq = args["Wq"], args["bq"]
    Wk, bk = args["Wk"], args["bk"]
    Wv, bv = args["Wv"], args["bv"]
    Wo, bo = args["Wo"], args["bo"]

    nc, mixed_tiles = _get_program(mask)
    in_maps = _prep_in_maps(x, mask, Wq, bq, Wk, bk, Wv, bv, Wo, bo,
                            mixed_tiles)
    res = bass_utils.run_bass_kernel_spmd(
        nc, in_maps, core_ids=list(range(NCORES)), trace=trace)
    out = _unshard(res.results, Wo, bv, bo)
    kernel.last_results = res
    return out


# revision 7
# speedup vs baseline: 1.1779x; 1.1779x over previous
"""Multi-head attention (B=2, S=2048, D=1024, H=16) on 8 Trainium2 cores.

Sharding: core = b*4 + g  ->  batch b (data parallel), head-group g of 4
heads (tensor parallel).  Each core computes a partial out^T = Wo_g^T @ Z_g
for its batch; the host sums the 4 partials per batch (the "all-reduce"),
transposes back and adds the (folded) output bias.

All activations flow feature-major on device (x^T, Q^T, K^T, scores^T) so
no on-device transposes are needed.  Matmuls run in bf16 with fp32 PSUM
accumulation.  Softmax skips the row-max pass (scores are bounded), gets
its denominator from a ones-column appended to V, and defers normalization
to after the attention*V matmul.  The reciprocal of the denominator is
broadcast across partitions with a rank-1 PE matmul (K=1) instead of a
DRAM round trip, and its instructions are emitted a few blocks late so
the in-order PE queue never stalls waiting on it.  Output partials are
written back in bf16 (the host accumulates in fp32).
"""

import numpy as np
import ml_dtypes

B, S, D, H = 2, 2048, 1024, 16
DK = D // H                  # 64
SCALE = 1.0 / np.sqrt(D)
NCORES = 8
GROUPS = 4                   # head-groups (tensor parallel)
HG = H // GROUPS             # 4 heads per group
DG = D // GROUPS             # 256 head dims per group
P = 128
KO = D // P                  # 8 contraction chunks for the projections
MO = DG // P                 # 2 row-chunks of Q^T/K^T (= head pairs)
NQ = 512                     # q tile width
QT = S // NQ                 # 4
ST = S // P                  # 16 key blocks / s chunks
BF16 = ml_dtypes.bfloat16

_cache = {}


def _classify_mask(mask):
    """Block structure of mask^T ([k, q] layout, P x NQ blocks).

    Returns (cls, qoff, mixed_idx, mixed_tiles, use_affine):
      cls[kt][qt]  : 0 all-masked, 1 all-kept, 2 mixed
      qoff[kt][qt] : leading all-masked columns (trim), 0 unless tril
      mixed_idx    : {(kt, qt): index into mixed_tiles}
      mixed_tiles  : np [n, P, NQ] bf16 0/1 tiles (empty when use_affine)
    """
    tril = np.tril(np.ones((S, S), dtype=mask.dtype))
    use_affine = bool(np.array_equal(mask, tril))
    cls = [[1] * QT for _ in range(ST)]
    qoff = [[0] * QT for _ in range(ST)]
    mixed_idx = {}
    tiles = []
    if use_affine:
        for kt in range(ST):
            k0 = kt * P
            for qt in range(QT):
                q0 = qt * NQ
                if k0 - q0 >= NQ:
                    cls[kt][qt] = 0
                elif k0 + P - 1 > q0:
                    cls[kt][qt] = 2
                    qoff[kt][qt] = min(max(k0 - q0, 0), NQ - P)
                # else: fully kept
    else:
        keepT = (mask != 0).T        # [k, q]
        for kt in range(ST):
            for qt in range(QT):
                blk = keepT[kt * P:(kt + 1) * P, qt * NQ:(qt + 1) * NQ]
                if not blk.any():
                    cls[kt][qt] = 0
                elif blk.all():
                    cls[kt][qt] = 1
                else:
                    cls[kt][qt] = 2
                    mixed_idx[(kt, qt)] = len(tiles)
                    tiles.append(blk.astype(BF16))
    mixed_tiles = (np.stack(tiles) if tiles else
                   np.zeros((0, P, NQ), dtype=BF16))
    return cls, qoff, mixed_idx, mixed_tiles, use_affine


def _build_program(cls, qoff, mixed_idx, n_mixed, use_affine):
    from contextlib import ExitStack
    import concourse.bass as bass
    import concourse.tile as tile
    import concourse.mybir as mybir
    from concourse import bacc
    from concourse.bass import ds, ts

    f32 = mybir.dt.float32
    bf16 = mybir.dt.bfloat16
    Exp = mybir.ActivationFunctionType.Exp

    nc = bacc.Bacc(None, target_bir_lowering=False, name="mha_tp")

    xT = nc.dram_tensor("xT", [D, S], bf16, kind="ExternalInput")
    wq = nc.dram_tensor("wq", [D, DG], bf16, kind="ExternalInput")
    wk = nc.dram_tensor("wk", [D, DG], bf16, kind="ExternalInput")
    wv = nc.dram_tensor("wv", [D, DG], bf16, kind="ExternalInput")
    wo = nc.dram_tensor("wo", [DG, D], bf16, kind="ExternalInput")
    bqk = nc.dram_tensor("bqk", [2, DG], f32, kind="ExternalInput")
    mm = (nc.dram_tensor("mmask", [n_mixed, P, NQ], bf16, kind="ExternalInput")
          if n_mixed else None)
    outT = nc.dram_tensor("outT", [D, S], bf16, kind="ExternalOutput")

    xTv = xT.ap().rearrange("(ko p) s -> p ko s", p=P)
    wqv = wq.ap().rearrange("(ko p) m -> p ko m", p=P)
    wkv = wk.ap().rearrange("(ko p) m -> p ko m", p=P)
    wvv = wv.ap().rearrange("(ko p) m -> p ko m", p=P)
    wov = wo.ap().rearrange("(zo p) n -> p zo n", p=P)
    bqkv = bqk.ap().rearrange("t (mo p) -> p t mo", p=P)
    outv = outT.ap().rearrange("(mo p) s -> p mo s", p=P)

    with tile.TileContext(nc) as tc, ExitStack() as ctx:
        const = ctx.enter_context(tc.tile_pool(name="const", bufs=1))

        # DMA order matters: the first QKV matmul needs wq + the first x
        # column slab; interleave them in small pieces so the lead-in is
        # paced by queue parallelism, not one big serial transfer.
        bias_sb = const.tile([P, 2, 2], f32)
        nc.sync.dma_start(bias_sb[:], bqkv)
        wq_sb = const.tile([P, KO, DG], bf16)
        x_sb = const.tile([P, KO, S], bf16)
        for ko in range(KO):
            nc.sync.dma_start(wq_sb[:, ko, :], wqv[:, ko, :])
            nc.sync.dma_start(x_sb[:, ko, ts(0, NQ)], xTv[:, ko, ts(0, NQ)])
        wk_sb = const.tile([P, KO, DG], bf16)
        for ko in range(KO):
            nc.sync.dma_start(wk_sb[:, ko, :], wkv[:, ko, :])
        wv_sb = const.tile([P, KO, DG], bf16)
        for ko in range(KO):
            nc.sync.dma_start(wv_sb[:, ko, :], wvv[:, ko, :])
        for ko in range(KO):
            nc.sync.dma_start(x_sb[:, ko, ts(1, NQ)], xTv[:, ko, ts(1, NQ)])
        mask_sb = None
        if n_mixed:
            mask_sb = const.tile([P, n_mixed, NQ], bf16)
            for i in range(n_mixed):
                nc.sync.dma_start(mask_sb[:, i, :], mm.ap()[i])
        wo_sb = const.tile([P, MO, D], bf16)
        for zo in range(MO):
            nc.sync.dma_start(wo_sb[:, zo, :], wov[:, zo, :])
        for qt in range(2, QT):
            for ko in range(KO):
                nc.sync.dma_start(x_sb[:, ko, ts(qt, NQ)],
                                  xTv[:, ko, ts(qt, NQ)])

        qT_sb = const.tile([P, MO, S], bf16)
        kT_sb = const.tile([P, MO, S], bf16)
        v_sb = const.tile([P, ST, HG, DK + 1], bf16)
        zT_sb = const.tile([P, MO, S], bf16)
        nc.gpsimd.memset(v_sb[:, :, :, DK:DK + 1], 1.0)
        # all-ones row used by the rank-1 reciprocal-broadcast matmul;
        # kept full-height so partition-64 slices exist.
        ones_sb = const.tile([P, DK], f32)
        nc.gpsimd.memset(ones_sb[:], 1.0)
        warm_sb = const.tile([P, DK], bf16)
        nc.gpsimd.memset(warm_sb[:], 0.0)

        with (
            tc.tile_pool(name="pqkv", bufs=2, space="PSUM") as pqkv,
            tc.tile_pool(name="ps_at", bufs=2, space="PSUM") as ps_at,
            tc.tile_pool(name="pz", bufs=1, space="PSUM") as pz,
            tc.tile_pool(name="work", bufs=8) as work,
            tc.tile_pool(name="rwork", bufs=3) as rwork,
        ):
            # keep the PE busy during the DMA lead-in so the HAM clock
            # gate is already released when the first real matmul issues
            zp_warm = pz.tile([P, 2, NQ], f32, tag="z")
            for _ in range(28):
                nc.tensor.matmul(zp_warm[0:DK, 0, 0:DK], warm_sb[:, :],
                                 warm_sb[:, :], start=True, stop=True)

            def av(zp, mo, prev, last):
                kt, pT, off, first = prev
                ret = None
                for h in (0, 1):
                    ret = nc.tensor.matmul(
                        zp[0:DK + 1, h, off:],
                        v_sb[:, kt, 2 * mo + h, :],
                        pT[:, h, off:],
                        start=first, stop=last)
                return ret

            def outproj_chunk(qt, mo8, split=1):
                nw = NQ // split
                for s in range(split):
                    o_ps = pqkv.tile([P, NQ], f32, tag="ps", name=f"o{mo8}")
                    for zo in range(MO):
                        nc.tensor.matmul(
                            o_ps[:, 0:nw], wo_sb[:, zo, ts(mo8, P)],
                            zT_sb[:, zo, ds(qt * NQ + s * nw, nw)],
                            start=(zo == 0), stop=(zo == MO - 1))
                    o_sb = work.tile([P, NQ], bf16, tag="osb")
                    if (mo8 + s) % 2 == 0:
                        nc.vector.tensor_copy(o_sb[:, 0:nw], o_ps[:, 0:nw])
                    else:
                        nc.scalar.copy(o_sb[:, 0:nw], o_ps[:, 0:nw])
                    nc.sync.dma_start(
                        outv[:, mo8, ds(qt * NQ + s * nw, nw)], o_sb[:, 0:nw])

            def qkv_slab(qt, mid_hook=None):
                for t, (w_sb, dst) in enumerate(((wq_sb, qT_sb),
                                                 (wk_sb, kT_sb))):
                    for mo in range(MO):
                        ps = pqkv.tile([P, NQ], f32, tag="ps")
                        for ko in range(KO):
                            nc.tensor.matmul(
                                ps, w_sb[:, ko, ts(mo, P)],
                                x_sb[:, ko, ts(qt, NQ)],
                                start=(ko == 0), stop=(ko == KO - 1))
                        nc.vector.tensor_scalar_add(
                            dst[:, mo, ts(qt, NQ)], ps,
                            bias_sb[:, t, mo:mo + 1])
                    if t == 0 and mid_hook is not None:
                        mid_hook()
                for so in range(HG * qt, HG * (qt + 1)):
                    ps = pqkv.tile([P, NQ], f32, tag="ps")
                    for ko in range(KO):
                        nc.tensor.matmul(
                            ps[:, :DG], x_sb[:, ko, ts(so, P)],
                            wv_sb[:, ko, :],
                            start=(ko == 0), stop=(ko == KO - 1))
                    nc.vector.tensor_copy(
                        v_sb[:, so, :, 0:DK],
                        ps[:, :DG].rearrange("p (h d) -> p h d", h=HG))

            # --- deferred softmax normalization ---------------------------
            # norm_a (emitted inline, right after the slab's last AV):
            #   reciprocal of the denominator row + copy of raw z out of
            #   PSUM (frees the bank).  norm_b (emitted a little later, so
            #   the PE queue has covering work): rank-1 broadcast matmul of
            #   the reciprocal + the normalizing multiplies.
            pending_nb = [None]

            def norm_a(mo, qt, zp, h1_on_act):
                # copy raw z (plus the denominator row DK) out of PSUM,
                # then a fast approximate reciprocal of that row.  The
                # copy frees the pz bank; exact reciprocal() costs
                # ~6.4ns/element and would stall the whole chain.
                zraw = rwork.tile([DK + 1, 2, NQ], f32, tag="zraw")
                if h1_on_act:
                    nc.vector.tensor_copy(zraw[:, 0, :], zp[:DK + 1, 0, :])
                    nc.scalar.copy(zraw[:, 1, :], zp[:DK + 1, 1, :])
                else:
                    nc.vector.tensor_copy(zraw[:], zp[:DK + 1, :, :])
                # reciprocal costs ~6.4ns/element of free size, so spread
                # the denominator row across all 128 partitions first
                # (small SBUF->SBUF DMAs), then gather back to one row for
                # the rank-1 broadcast matmul.
                NJ = 2 * NQ // P
                d_sp = rwork.tile([P, NJ], f32, tag="dsp")
                nc.scalar.dma_start(d_sp[:], zraw[DK:DK + 1, :, :])
                r_sp = rwork.tile([P, NJ], f32, tag="rsp")
                nc.vector.reciprocal(r_sp[:], d_sp[:])
                r_row = rwork.tile([DK + 1, 2, NQ], f32, tag="rrow")
                nc.scalar.dma_start(r_row[DK:DK + 1, :, :], r_sp[:])
                pending_nb[0] = (mo, qt, zraw, r_row)

            def norm_b():
                if pending_nb[0] is None:
                    return
                mo, qt, zraw, r_row = pending_nb[0]
                pending_nb[0] = None
                for h in (0, 1):
                    rb = pqkv.tile([P, NQ], f32, tag="ps", name=f"rb{h}")
                    nc.tensor.matmul(
                        rb[0:DK, :], ones_sb[DK:DK + 1, :],
                        r_row[DK:DK + 1, h, :], start=True, stop=True)
                    if h == 0:
                        nc.vector.tensor_mul(
                            zT_sb[0:DK, mo, ts(qt, NQ)], zraw[0:DK, 0, :],
                            rb[0:DK, :])
                    else:
                        zn_tmp = rwork.tile([DK, NQ], bf16, tag="zt")
                        nc.vector.tensor_mul(zn_tmp[:], zraw[0:DK, 1, :],
                                             rb[0:DK, :])
                        nc.sync.dma_start(zT_sb[DK:P, mo, ts(qt, NQ)],
                                          zn_tmp[:])

            if not use_affine:
                # a general mask may attend beyond block qt, so all K/V
                # slabs must exist before any attention starts
                for qt in range(QT):
                    qkv_slab(qt)

            proc = list(range(QT))
            emitted = 0
            prev_qt = None
            for qt in proc:
                q0 = qt * NQ
                if use_affine:
                    # attention(qt) only needs k blocks <= qt, so emit QKV
                    # slabs lazily just ahead of it; the previous slab's
                    # last norm_b lands between the Q and K projections.
                    while emitted <= qt:
                        qkv_slab(emitted,
                                 mid_hook=norm_b if emitted == qt else None)
                        emitted += 1
                else:
                    norm_b()

                # -- attention over k blocks of this slab -----------------
                for mo in range(MO):
                    kts = [kt for kt in range(ST) if cls[kt][qt] != 0]
                    if not kts:
                        nc.vector.memset(zT_sb[:, mo, ts(qt, NQ)], 0.0)
                        continue
                    # out-projection of the previously processed q slab
                    # rides along inside this kt stream (mo==1); hold back
                    # 3 chunks of the penultimate slab as covering PE work
                    # for the final normalization chain
                    outq = []
                    if mo == 1 and prev_qt is not None:
                        nch = 8 if qt != QT - 1 else 4
                        outq = [(prev_qt, j) for j in range(nch)]
                    per = -(-len(outq) // len(kts)) if outq else 0
                    zp = pz.tile([P, 2, NQ], f32, tag="z")
                    prev = None
                    for i, kt in enumerate(kts):
                        k0 = kt * P
                        off = qoff[kt][qt]
                        w = NQ - off
                        pT = work.tile([P, 2, NQ], bf16, tag="pT")
                        s_ps = ps_at.tile([P, 2, NQ], f32, tag="s")
                        for h in (0, 1):
                            hp = slice(h * DK, (h + 1) * DK)
                            nc.tensor.matmul(
                                s_ps[:, h, off:],
                                kT_sb[hp, mo, ts(kt, P)],
                                qT_sb[hp, mo, ds(q0 + off, w)],
                                start=True, stop=True,
                                tile_position=(h * DK, 0))
                        nc.scalar.activation(
                            pT[:, :, off:], s_ps[:, :, off:], Exp)
                        if cls[kt][qt] == 2:
                            if use_affine:
                                nc.gpsimd.affine_select(
                                    out=pT[:, :, off:],
                                    in_=pT[:, :, off:],
                                    compare_op=mybir.AluOpType.is_ge,
                                    fill=0.0,
                                    base=q0 + off - k0,
                                    channel_multiplier=-1,
                                    pattern=[[0, 2], [1, w]])
                            else:
                                nc.vector.tensor_mul(
                                    pT[:, :, off:], pT[:, :, off:],
                                    mask_sb[:, mixed_idx[(kt, qt)], None,
                                            off:].to_broadcast((P, 2, w)))
                        if prev is not None:
                            av(zp, mo, prev, last=False)
                        prev = (kt, pT, off, i == 0)
                        if mo == 1 and i == 2:
                            norm_b()   # this slab's mo=0 normalization
                        for qtd, j in outq[per * i: per * (i + 1)]:
                            outproj_chunk(qtd, j)
                    for qtd, j in outq[per * len(kts):]:
                        outproj_chunk(qtd, j)
                    av(zp, mo, prev, last=True)
                    norm_a(mo, qt, zp,
                           h1_on_act=(qt == proc[-1] and mo == MO - 1))
                prev_qt = qt

            # -- tail: held-back chunks cover the final norm chain --------
            if QT >= 2:
                outproj_chunk(QT - 2, 4)
                outproj_chunk(QT - 2, 5)
                outproj_chunk(QT - 2, 6)
                norm_b()
                outproj_chunk(QT - 2, 7)
            else:
                norm_b()
            for mo8 in range(D // P):
                outproj_chunk(prev_qt, mo8, split=2)

    return nc


def _get_program(mask):
    cls, qoff, mixed_idx, mixed_tiles, use_affine = _classify_mask(mask)
    key = (use_affine,
           tuple(tuple(r) for r in cls),
           tuple(tuple(r) for r in qoff))
    if key not in _cache:
        nc = _build_program(cls, qoff, mixed_idx, len(mixed_tiles), use_affine)
        nc.compile()
        _cache[key] = nc
    return _cache[key], mixed_tiles


def _prep_in_maps(x, mask, Wq, bq, Wk, bk, Wv, bv, Wo, bo, mixed_tiles):
    xT = [np.ascontiguousarray(x[b].T).astype(BF16) for b in range(B)]
    in_maps = []
    for core in range(NCORES):
        b, g = divmod(core, GROUPS)
        c0, c1 = g * DG, (g + 1) * DG
        im = {
            "xT": xT[b],
            "wq": np.ascontiguousarray(Wq[:, c0:c1] * SCALE).astype(BF16),
            "wk": np.ascontiguousarray(Wk[:, c0:c1]).astype(BF16),
            "wv": np.ascontiguousarray(Wv[:, c0:c1]).astype(BF16),
            "wo": np.ascontiguousarray(Wo[c0:c1, :]).astype(BF16),
            "bqk": np.ascontiguousarray(
                np.stack([bq[c0:c1] * SCALE, bk[c0:c1]])).astype(np.float32),
        }
        if len(mixed_tiles):
            im["mmask"] = mixed_tiles
        in_maps.append(im)
    return in_maps


def _unshard(results, Wo, bv, bo):
    bo_eff = (bo.astype(np.float32)
              + bv.astype(np.float32) @ Wo.astype(np.float32))
    out = np.empty((B, S, D), np.float32)
    for b in range(B):
        acc = results[b * GROUPS]["outT"].astype(np.float32)
        for g in range(1, GROUPS):
            acc += results[b * GROUPS + g]["outT"]
        out[b] = acc.T + bo_eff
    return out


def kernel(trace=False, **inputs):
    from concourse import bass_utils

    args = {k: np.asarray(v) for k, v in inputs.items()}
    x, mask = args["x"], args["mask"]
    Wq, bq = args["Wq"], args["bq"]
    Wk, bk = args["Wk"], args["bk"]
    Wv, bv = args["Wv"], args["bv"]
    Wo, bo = args["Wo"], args["bo"]

    nc, mixed_tiles = _get_program(mask)
    in_maps = _prep_in_maps(x, mask, Wq, bq, Wk, bk, Wv, bv, Wo, bo,
                            mixed_tiles)
    res = bass_utils.run_bass_kernel_spmd(
        nc, in_maps, core_ids=list(range(NCORES)), trace=trace)
    out = _unshard(res.results, Wo, bv, bo)
    kernel.last_results = res
    return out


# revision 12
# speedup vs baseline: 1.2779x; 1.0848x over previous
"""Multi-head attention (B=2, S=2048, D=1024, H=16) on 8 Trainium2 cores.

Sharding: core = b*4 + g  ->  batch b (data parallel), head-group g of 4
heads (tensor parallel).  Each core computes a partial out^T = Wo_g^T @ Z_g
for its batch; the host sums the 4 partials per batch (the "all-reduce"),
transposes back and adds the (folded) output bias.

All activations flow feature-major on device (x^T, Q^T, K^T, scores^T) so
no on-device transposes are needed.  Matmuls run in bf16 with fp32 PSUM
accumulation.  Softmax skips the row-max pass (scores are bounded), gets
its denominator from a ones-column appended to V, and defers normalization
to after the attention*V matmul.  The reciprocal of the denominator is
broadcast across partitions with a rank-1 PE matmul (K=1) instead of a
DRAM round trip, and its instructions are emitted a few blocks late so
the in-order PE queue never stalls waiting on it.  Output partials are
written back in bf16 (the host accumulates in fp32).
"""

import numpy as np
import ml_dtypes

B, S, D, H = 2, 2048, 1024, 16
DK = D // H                  # 64
SCALE = 1.0 / np.sqrt(D)
NCORES = 8
GROUPS = 4                   # head-groups (tensor parallel)
HG = H // GROUPS             # 4 heads per group
DG = D // GROUPS             # 256 head dims per group
P = 128
KO = D // P                  # 8 contraction chunks for the projections
MO = DG // P                 # 2 row-chunks of Q^T/K^T (= head pairs)
NQ = 512                     # q tile width
QT = S // NQ                 # 4
ST = S // P                  # 16 key blocks / s chunks
BF16 = ml_dtypes.bfloat16

_cache = {}


def _classify_mask(mask):
    """Block structure of mask^T ([k, q] layout, P x NQ blocks).

    Returns (cls, qoff, mixed_idx, mixed_tiles, use_affine):
      cls[kt][qt]  : 0 all-masked, 1 all-kept, 2 mixed
      qoff[kt][qt] : leading all-masked columns (trim), 0 unless tril
      mixed_idx    : {(kt, qt): index into mixed_tiles}
      mixed_tiles  : np [n, P, NQ] bf16 0/1 tiles (empty when use_affine)
    """
    tril = np.tril(np.ones((S, S), dtype=mask.dtype))
    use_affine = bool(np.array_equal(mask, tril))
    cls = [[1] * QT for _ in range(ST)]
    qoff = [[0] * QT for _ in range(ST)]
    mixed_idx = {}
    tiles = []
    if use_affine:
        for kt in range(ST):
            k0 = kt * P
            for qt in range(QT):
                q0 = qt * NQ
                if k0 - q0 >= NQ:
                    cls[kt][qt] = 0
                elif k0 + P - 1 > q0:
                    cls[kt][qt] = 2
                    qoff[kt][qt] = min(max(k0 - q0, 0), NQ - P)
                # else: fully kept
    else:
        keepT = (mask != 0).T        # [k, q]
        for kt in range(ST):
            for qt in range(QT):
                blk = keepT[kt * P:(kt + 1) * P, qt * NQ:(qt + 1) * NQ]
                if not blk.any():
                    cls[kt][qt] = 0
                elif blk.all():
                    cls[kt][qt] = 1
                else:
                    cls[kt][qt] = 2
                    mixed_idx[(kt, qt)] = len(tiles)
                    tiles.append(blk.astype(BF16))
    mixed_tiles = (np.stack(tiles) if tiles else
                   np.zeros((0, P, NQ), dtype=BF16))
    return cls, qoff, mixed_idx, mixed_tiles, use_affine


def _build_program(cls, qoff, mixed_idx, n_mixed, use_affine):
    from contextlib import ExitStack
    import concourse.bass as bass
    import concourse.tile as tile
    import concourse.mybir as mybir
    from concourse import bacc
    from concourse.bass import ds, ts

    f32 = mybir.dt.float32
    bf16 = mybir.dt.bfloat16
    Exp = mybir.ActivationFunctionType.Exp

    nc = bacc.Bacc(None, target_bir_lowering=False, name="mha_tp")

    xT = nc.dram_tensor("xT", [D, S], bf16, kind="ExternalInput")
    wq = nc.dram_tensor("wq", [D, DG], bf16, kind="ExternalInput")
    wk = nc.dram_tensor("wk", [D, DG], bf16, kind="ExternalInput")
    wv = nc.dram_tensor("wv", [D, DG], bf16, kind="ExternalInput")
    wo = nc.dram_tensor("wo", [DG, D], bf16, kind="ExternalInput")
    bqk = nc.dram_tensor("bqk", [2, DG], f32, kind="ExternalInput")
    mm = (nc.dram_tensor("mmask", [n_mixed, P, NQ], bf16, kind="ExternalInput")
          if n_mixed else None)
    outT = nc.dram_tensor("outT", [D, S], bf16, kind="ExternalOutput")

    xTv = xT.ap().rearrange("(ko p) s -> p ko s", p=P)
    wqv = wq.ap().rearrange("(ko p) m -> p ko m", p=P)
    wkv = wk.ap().rearrange("(ko p) m -> p ko m", p=P)
    wvv = wv.ap().rearrange("(ko p) m -> p ko m", p=P)
    wov = wo.ap().rearrange("(zo p) n -> p zo n", p=P)
    bqkv = bqk.ap().rearrange("t (mo p) -> p t mo", p=P)
    outv = outT.ap().rearrange("(mo p) s -> p mo s", p=P)

    with tile.TileContext(nc) as tc, ExitStack() as ctx:
        const = ctx.enter_context(tc.tile_pool(name="const", bufs=1))

        # DMA order matters: the first QKV matmul needs wq + the first x
        # column slab; interleave them in small pieces so the lead-in is
        # paced by queue parallelism, not one big serial transfer.
        bias_sb = const.tile([P, 2, 2], f32)
        nc.sync.dma_start(bias_sb[:], bqkv)
        wq_sb = const.tile([P, KO, DG], bf16)
        x_sb = const.tile([P, KO, S], bf16)
        for ko in range(KO):
            nc.sync.dma_start(wq_sb[:, ko, :], wqv[:, ko, :])
            nc.sync.dma_start(x_sb[:, ko, ts(0, NQ)], xTv[:, ko, ts(0, NQ)])
        wk_sb = const.tile([P, KO, DG], bf16)
        for ko in range(KO):
            nc.sync.dma_start(wk_sb[:, ko, :], wkv[:, ko, :])
        wv_sb = const.tile([P, KO, DG], bf16)
        for ko in range(KO):
            nc.sync.dma_start(wv_sb[:, ko, :], wvv[:, ko, :])
        for ko in range(KO):
            nc.sync.dma_start(x_sb[:, ko, ts(1, NQ)], xTv[:, ko, ts(1, NQ)])
        mask_sb = None
        if n_mixed:
            mask_sb = const.tile([P, n_mixed, NQ], bf16)
            for i in range(n_mixed):
                nc.sync.dma_start(mask_sb[:, i, :], mm.ap()[i])
        wo_sb = const.tile([P, MO, D], bf16)
        for zo in range(MO):
            nc.sync.dma_start(wo_sb[:, zo, :], wov[:, zo, :])
        for qt in range(2, QT):
            for ko in range(KO):
                nc.sync.dma_start(x_sb[:, ko, ts(qt, NQ)],
                                  xTv[:, ko, ts(qt, NQ)])

        qT_sb = const.tile([P, MO, S], bf16)
        kT_sb = const.tile([P, MO, S], bf16)
        v_sb = const.tile([P, ST, HG, DK + 1], bf16)
        zT_sb = const.tile([P, MO, S], bf16)
        nc.gpsimd.memset(v_sb[:, :, :, DK:DK + 1], 1.0)
        # all-ones row used by the rank-1 reciprocal-broadcast matmul;
        # kept full-height so partition-64 slices exist.
        ones_sb = const.tile([P, DK], bf16)
        nc.gpsimd.memset(ones_sb[:], 1.0)
        warm_sb = const.tile([P, DK], bf16)
        nc.gpsimd.memset(warm_sb[:], 0.0)

        with (
            tc.tile_pool(name="pqkv", bufs=2, space="PSUM") as pqkv,
            tc.tile_pool(name="ps_at", bufs=2, space="PSUM") as ps_at,
            tc.tile_pool(name="pz", bufs=1, space="PSUM") as pz,
            tc.tile_pool(name="work", bufs=8) as work,
            tc.tile_pool(name="rwork", bufs=3) as rwork,
        ):
            # keep the PE busy during the DMA lead-in so the HAM clock
            # gate is already released when the first real matmul issues
            zp_warm = pz.tile([P, 2, NQ], f32, tag="z")
            for _ in range(28):
                nc.tensor.matmul(zp_warm[0:DK, 0, 0:DK], warm_sb[:, :],
                                 warm_sb[:, :], start=True, stop=True)

            def av(zp, mo, prev, last):
                kt, pT, off, first = prev
                ret = None
                for h in (0, 1):
                    ret = nc.tensor.matmul(
                        zp[0:DK + 1, h, off:],
                        v_sb[:, kt, 2 * mo + h, :],
                        pT[:, h, off:],
                        start=first, stop=last)
                return ret

            def outproj_chunk(qt, mo8, split=1):
                nw = NQ // split
                for s in range(split):
                    o_ps = pqkv.tile([P, NQ], f32, tag="ps", name=f"o{mo8}")
                    for zo in range(MO):
                        nc.tensor.matmul(
                            o_ps[:, 0:nw], wo_sb[:, zo, ts(mo8, P)],
                            zT_sb[:, zo, ds(qt * NQ + s * nw, nw)],
                            start=(zo == 0), stop=(zo == MO - 1))
                    o_sb = work.tile([P, NQ], bf16, tag="osb")
                    if (mo8 + s) % 2 == 0:
                        nc.vector.tensor_copy(o_sb[:, 0:nw], o_ps[:, 0:nw])
                    else:
                        nc.scalar.copy(o_sb[:, 0:nw], o_ps[:, 0:nw])
                    nc.sync.dma_start(
                        outv[:, mo8, ds(qt * NQ + s * nw, nw)], o_sb[:, 0:nw])

            def qkv_slab(qt, mid_hook=None):
                for t, (w_sb, dst) in enumerate(((wq_sb, qT_sb),
                                                 (wk_sb, kT_sb))):
                    for mo in range(MO):
                        ps = pqkv.tile([P, NQ], f32, tag="ps")
                        for ko in range(KO):
                            nc.tensor.matmul(
                                ps, w_sb[:, ko, ts(mo, P)],
                                x_sb[:, ko, ts(qt, NQ)],
                                start=(ko == 0), stop=(ko == KO - 1))
                        nc.vector.tensor_scalar_add(
                            dst[:, mo, ts(qt, NQ)], ps,
                            bias_sb[:, t, mo:mo + 1])
                    if t == 0 and mid_hook is not None:
                        mid_hook()
                for so in range(HG * qt, HG * (qt + 1)):
                    ps = pqkv.tile([P, NQ], f32, tag="ps")
                    for ko in range(KO):
                        nc.tensor.matmul(
                            ps[:, :DG], x_sb[:, ko, ts(so, P)],
                            wv_sb[:, ko, :],
                            start=(ko == 0), stop=(ko == KO - 1))
                    nc.vector.tensor_copy(
                        v_sb[:, so, :, 0:DK],
                        ps[:, :DG].rearrange("p (h d) -> p h d", h=HG))

            # --- deferred softmax normalization ---------------------------
            # norm_a (emitted inline, right after the slab's last AV):
            #   reciprocal of the denominator row + copy of raw z out of
            #   PSUM (frees the bank).  norm_b (emitted a little later, so
            #   the PE queue has covering work): rank-1 broadcast matmul of
            #   the reciprocal + the normalizing multiplies.
            pending_nb = [None]

            def norm_a(mo, qt, zp, h1_on_act):
                # copy raw z (plus the denominator row DK) out of PSUM,
                # then a fast approximate reciprocal of that row.  The
                # copy frees the pz bank; exact reciprocal() costs
                # ~6.4ns/element and would stall the whole chain.
                zraw = rwork.tile([DK + 1, 2, NQ], f32, tag="zraw")
                if h1_on_act:
                    nc.vector.tensor_copy(zraw[:, 0, :], zp[:DK + 1, 0, :])
                    nc.scalar.copy(zraw[:, 1, :], zp[:DK + 1, 1, :])
                else:
                    nc.vector.tensor_copy(zraw[:], zp[:DK + 1, :, :])
                # reciprocal costs ~6.4ns/element of free size, so spread
                # the denominator row across all 128 partitions first
                # (small SBUF->SBUF DMAs), then gather back to one row for
                # the rank-1 broadcast matmul.
                # Issue the spread/gather DMAs from the SP queue, NOT the
                # ACT queue: a DMA waiting on its input blocks the issuing
                # engine's queue head, and ACT must keep streaming the
                # next head-pair's EXPs.
                NJ = 2 * NQ // P
                d_sp = rwork.tile([P, NJ], f32, tag="dsp")
                nc.sync.dma_start(d_sp[:], zraw[DK:DK + 1, :, :])
                r_sp = rwork.tile([P, NJ], bf16, tag="rsp")
                with nc.allow_low_precision("softmax denom tolerates bf16"):
                    nc.vector.reciprocal(r_sp[:], d_sp[:])
                r_row = rwork.tile([DK + 1, 2, NQ], bf16, tag="rrow")
                nc.sync.dma_start(r_row[DK:DK + 1, :, :], r_sp[:])
                pending_nb[0] = (mo, qt, zraw, r_row)

            def norm_b():
                if pending_nb[0] is None:
                    return
                mo, qt, zraw, r_row = pending_nb[0]
                pending_nb[0] = None
                for h in (0, 1):
                    rb = pqkv.tile([P, NQ], f32, tag="ps", name=f"rb{h}")
                    nc.tensor.matmul(
                        rb[0:DK, :], ones_sb[DK:DK + 1, :],
                        r_row[DK:DK + 1, h, :], start=True, stop=True)
                    if h == 0:
                        nc.vector.tensor_mul(
                            zT_sb[0:DK, mo, ts(qt, NQ)], zraw[0:DK, 0, :],
                            rb[0:DK, :])
                    else:
                        zn_tmp = rwork.tile([DK, NQ], bf16, tag="zt")
                        nc.vector.tensor_mul(zn_tmp[:], zraw[0:DK, 1, :],
                                             rb[0:DK, :])
                        nc.sync.dma_start(zT_sb[DK:P, mo, ts(qt, NQ)],
                                          zn_tmp[:])

            if not use_affine:
                # a general mask may attend beyond block qt, so all K/V
                # slabs must exist before any attention starts
                for qt in range(QT):
                    qkv_slab(qt)

            proc = list(range(QT))
            emitted = 0
            prev_qt = None
            for qt in proc:
                q0 = qt * NQ
                if use_affine:
                    # attention(qt) only needs k blocks <= qt, so emit QKV
                    # slabs lazily just ahead of it; the previous slab's
                    # last norm_b lands between the Q and K projections.
                    while emitted <= qt:
                        qkv_slab(emitted,
                                 mid_hook=norm_b if emitted == qt else None)
                        emitted += 1
                else:
                    norm_b()

                # -- attention over k blocks of this slab -----------------
                for mo in range(MO):
                    kts = [kt for kt in range(ST) if cls[kt][qt] != 0]
                    if not kts:
                        nc.vector.memset(zT_sb[:, mo, ts(qt, NQ)], 0.0)
                        continue
                    # out-projection of the previously processed q slab
                    # rides along inside this kt stream (mo==1); hold back
                    # 3 chunks of the penultimate slab as covering PE work
                    # for the final normalization chain
                    outq = []
                    if mo == 1 and prev_qt is not None:
                        nch = 8 if qt != QT - 1 else 3
                        outq = [(prev_qt, j) for j in range(nch)]
                    per = -(-len(outq) // len(kts)) if outq else 0
                    zp = pz.tile([P, 2, NQ], f32, tag="z")
                    prev = None
                    for i, kt in enumerate(kts):
                        k0 = kt * P
                        off = qoff[kt][qt]
                        w = NQ - off
                        pT = work.tile([P, 2, NQ], bf16, tag="pT")
                        s_ps = ps_at.tile([P, 2, NQ], f32, tag="s")
                        for h in (0, 1):
                            hp = slice(h * DK, (h + 1) * DK)
                            nc.tensor.matmul(
                                s_ps[:, h, off:],
                                kT_sb[hp, mo, ts(kt, P)],
                                qT_sb[hp, mo, ds(q0 + off, w)],
                                start=True, stop=True,
                                tile_position=(h * DK, 0))
                        nc.scalar.activation(
                            pT[:, :, off:], s_ps[:, :, off:], Exp)
                        if cls[kt][qt] == 2:
                            if use_affine:
                                nc.gpsimd.affine_select(
                                    out=pT[:, :, off:],
                                    in_=pT[:, :, off:],
                                    compare_op=mybir.AluOpType.is_ge,
                                    fill=0.0,
                                    base=q0 + off - k0,
                                    channel_multiplier=-1,
                                    pattern=[[0, 2], [1, w]])
                            else:
                                nc.vector.tensor_mul(
                                    pT[:, :, off:], pT[:, :, off:],
                                    mask_sb[:, mixed_idx[(kt, qt)], None,
                                            off:].to_broadcast((P, 2, w)))
                        if prev is not None:
                            av(zp, mo, prev, last=False)
                        prev = (kt, pT, off, i == 0)
                        if mo == 1 and i == 2:
                            norm_b()   # this slab's mo=0 normalization
                        for qtd, j in outq[per * i: per * (i + 1)]:
                            outproj_chunk(qtd, j)
                    for qtd, j in outq[per * len(kts):]:
                        outproj_chunk(qtd, j)
                    av(zp, mo, prev, last=True)
                    norm_a(mo, qt, zp,
                           h1_on_act=(qt == proc[-1] and mo == MO - 1))
                prev_qt = qt

            # -- tail: held-back chunks cover the final norm chain --------
            if QT >= 2:
                outproj_chunk(QT - 2, 3)
                outproj_chunk(QT - 2, 4)
                outproj_chunk(QT - 2, 5)
                outproj_chunk(QT - 2, 6)
                norm_b()
                outproj_chunk(QT - 2, 7)
            else:
                norm_b()
            for mo8 in range(D // P):
                outproj_chunk(prev_qt, mo8, split=2)

    return nc


def _get_program(mask):
    cls, qoff, mixed_idx, mixed_tiles, use_affine = _classify_mask(mask)
    key = (use_affine,
           tuple(tuple(r) for r in cls),
           tuple(tuple(r) for r in qoff))
    if key not in _cache:
        nc = _build_program(cls, qoff, mixed_idx, len(mixed_tiles), use_affine)
        nc.compile()
        _cache[key] = nc
    return _cache[key], mixed_tiles


def _prep_in_maps(x, mask, Wq, bq, Wk, bk, Wv, bv, Wo, bo, mixed_tiles):
    xT = [np.ascontiguousarray(x[b].T).astype(BF16) for b in range(B)]
    in_maps = []
    for core in range(NCORES):
        b, g = divmod(core, GROUPS)
        c0, c1 = g * DG, (g + 1) * DG
        im = {
            "xT": xT[b],
            "wq": np.ascontiguousarray(Wq[:, c0:c1] * SCALE).astype(BF16),
            "wk": np.ascontiguousarray(Wk[:, c0:c1]).astype(BF16),
            "wv": np.ascontiguousarray(Wv[:, c0:c1]).astype(BF16),
            "wo": np.ascontiguousarray(Wo[c0:c1, :]).astype(BF16),
            "bqk": np.ascontiguousarray(
                np.stack([bq[c0:c1] * SCALE, bk[c0:c1]])).astype(np.float32),
        }
        if len(mixed_tiles):
            im["mmask"] = mixed_tiles
        in_maps.append(im)
    return in_maps


def _unshard(results, Wo, bv, bo):
    bo_eff = (bo.astype(np.float32)
              + bv.astype(np.float32) @ Wo.astype(np.float32))
    out = np.empty((B, S, D), np.float32)
    for b in range(B):
        acc = results[b * GROUPS]["outT"].astype(np.float32)
        for g in range(1, GROUPS):
            acc += results[b * GROUPS + g]["outT"]
        out[b] = acc.T + bo_eff
    return out


def kernel(trace=False, **inputs):
    from concourse import bass_utils

    args = {k: np.asarray(v) for k, v in inputs.items()}
    x, mask = args["x"], args["mask"]
    Wq, bq = args["Wq"], args["bq"]
    Wk, bk = args["Wk"], args["bk"]
    Wv, bv = args["Wv"], args["bv"]
    Wo, bo = args["Wo"], args["bo"]

    nc, mixed_tiles = _get_program(mask)
    in_maps = _prep_in_maps(x, mask, Wq, bq, Wk, bk, Wv, bv, Wo, bo,
                            mixed_tiles)
    res = bass_utils.run_bass_kernel_spmd(
        nc, in_maps, core_ids=list(range(NCORES)), trace=trace)
    out = _unshard(res.results, Wo, bv, bo)
    kernel.last_results = res
    return out


# revision 18
# speedup vs baseline: 1.3754x; 1.0764x over previous
"""Multi-head attention (B=2, S=2048, D=1024, H=16) on 8 Trainium2 cores.

Sharding: core = b*4 + g  ->  batch b (data parallel), head-group g of 4
heads (tensor parallel).  Each core computes a partial out^T = Wo_g^T @ Z_g
for its batch; the host sums the 4 partials per batch (the "all-reduce"),
transposes back and adds the (folded) output bias.

All activations flow feature-major on device (x^T, Q^T, K^T, scores^T) so
no on-device transposes are needed.  Matmuls run in bf16 with fp32 PSUM
accumulation.  Softmax skips the row-max pass (scores are bounded), gets
its denominator from a ones-column appended to V, and defers normalization
to after the attention*V matmul.  The reciprocal of the denominator is
broadcast across partitions with a rank-1 PE matmul (K=1) instead of a
DRAM round trip, and its instructions are emitted a few blocks late so
the in-order PE queue never stalls waiting on it.  Output partials are
written back in bf16 (the host accumulates in fp32).
"""

import numpy as np
import ml_dtypes

B, S, D, H = 2, 2048, 1024, 16
DK = D // H                  # 64
SCALE = 1.0 / np.sqrt(D)
NCORES = 8
GROUPS = 4                   # head-groups (tensor parallel)
HG = H // GROUPS             # 4 heads per group
DG = D // GROUPS             # 256 head dims per group
P = 128
KO = D // P                  # 8 contraction chunks for the projections
MO = DG // P                 # 2 row-chunks of Q^T/K^T (= head pairs)
NQ = 512                     # q tile width
QT = S // NQ                 # 4
ST = S // P                  # 16 key blocks / s chunks
BF16 = ml_dtypes.bfloat16

_cache = {}


def _classify_mask(mask):
    """Block structure of mask^T ([k, q] layout, P x NQ blocks).

    Returns (cls, qoff, mixed_idx, mixed_tiles, use_affine):
      cls[kt][qt]  : 0 all-masked, 1 all-kept, 2 mixed
      qoff[kt][qt] : leading all-masked columns (trim), 0 unless tril
      mixed_idx    : {(kt, qt): index into mixed_tiles}
      mixed_tiles  : np [n, P, NQ] bf16 0/1 tiles (empty when use_affine)
    """
    tril = np.tril(np.ones((S, S), dtype=mask.dtype))
    use_affine = bool(np.array_equal(mask, tril))
    cls = [[1] * QT for _ in range(ST)]
    qoff = [[0] * QT for _ in range(ST)]
    mixed_idx = {}
    tiles = []
    if use_affine:
        for kt in range(ST):
            k0 = kt * P
            for qt in range(QT):
                q0 = qt * NQ
                if k0 - q0 >= NQ:
                    cls[kt][qt] = 0
                elif k0 + P - 1 > q0:
                    cls[kt][qt] = 2
                    qoff[kt][qt] = min(max(k0 - q0, 0), NQ - P)
                # else: fully kept
    else:
        keepT = (mask != 0).T        # [k, q]
        for kt in range(ST):
            for qt in range(QT):
                blk = keepT[kt * P:(kt + 1) * P, qt * NQ:(qt + 1) * NQ]
                if not blk.any():
                    cls[kt][qt] = 0
                elif blk.all():
                    cls[kt][qt] = 1
                else:
                    cls[kt][qt] = 2
                    mixed_idx[(kt, qt)] = len(tiles)
                    tiles.append(blk.astype(BF16))
    mixed_tiles = (np.stack(tiles) if tiles else
                   np.zeros((0, P, NQ), dtype=BF16))
    return cls, qoff, mixed_idx, mixed_tiles, use_affine


def _build_program(cls, qoff, mixed_idx, n_mixed, use_affine):
    from contextlib import ExitStack
    import concourse.bass as bass
    import concourse.tile as tile
    import concourse.mybir as mybir
    from concourse import bacc
    from concourse.bass import ds, ts

    f32 = mybir.dt.float32
    bf16 = mybir.dt.bfloat16
    Exp = mybir.ActivationFunctionType.Exp

    nc = bacc.Bacc(None, target_bir_lowering=False, name="mha_tp")

    xT = nc.dram_tensor("xT", [D, S], bf16, kind="ExternalInput")
    wq = nc.dram_tensor("wq", [D, DG], bf16, kind="ExternalInput")
    wk = nc.dram_tensor("wk", [D, DG], bf16, kind="ExternalInput")
    wv = nc.dram_tensor("wv", [D, DG], bf16, kind="ExternalInput")
    wo = nc.dram_tensor("wo", [DG, D], bf16, kind="ExternalInput")
    bqk = nc.dram_tensor("bqk", [2, DG], f32, kind="ExternalInput")
    mm = (nc.dram_tensor("mmask", [n_mixed, P, NQ], bf16, kind="ExternalInput")
          if n_mixed else None)
    outT = nc.dram_tensor("outT", [D, S], bf16, kind="ExternalOutput")

    xTv = xT.ap().rearrange("(ko p) s -> p ko s", p=P)
    wqv = wq.ap().rearrange("(ko p) m -> p ko m", p=P)
    wkv = wk.ap().rearrange("(ko p) m -> p ko m", p=P)
    wvv = wv.ap().rearrange("(ko p) m -> p ko m", p=P)
    wov = wo.ap().rearrange("(zo p) n -> p zo n", p=P)
    bqkv = bqk.ap().rearrange("t (mo p) -> p t mo", p=P)
    outv = outT.ap().rearrange("(mo p) s -> p mo s", p=P)

    with tile.TileContext(nc) as tc, ExitStack() as ctx:
        const = ctx.enter_context(tc.tile_pool(name="const", bufs=1))

        # Each dma_start occupies its issuing sequencer for ~0.7us, so the
        # NUMBER of DMA instructions per queue matters as much as bytes.
        # SP carries wq + x (what the first matmuls need, finest pieces
        # first); ACT carries the rest of the weights during the lead-in
        # while it has nothing else to do.
        wq_sb = const.tile([P, KO, DG], bf16)
        x_sb = const.tile([P, KO, S], bf16)
        nc.sync.dma_start(wq_sb[:], wqv)
        for ko in range(KO):
            nc.sync.dma_start(x_sb[:, ko, ts(0, NQ)], xTv[:, ko, ts(0, NQ)])
        for qt in range(1, QT):
            nc.sync.dma_start(x_sb[:, :, ts(qt, NQ)], xTv[:, :, ts(qt, NQ)])
        bias_sb = const.tile([P, 2, 2], f32)
        nc.scalar.dma_start(bias_sb[:], bqkv)
        wk_sb = const.tile([P, KO, DG], bf16)
        nc.scalar.dma_start(wk_sb[:], wkv)
        wv_sb = const.tile([P, KO, DG], bf16)
        nc.scalar.dma_start(wv_sb[:], wvv)
        wo_sb = const.tile([P, MO, D], bf16)
        nc.scalar.dma_start(wo_sb[:], wov)
        mask_sb = None
        if n_mixed:
            mask_sb = const.tile([P, n_mixed, NQ], bf16)
            nc.scalar.dma_start(mask_sb[:], mm.ap())

        qT_sb = const.tile([P, MO, S], bf16)
        kT_sb = const.tile([P, MO, S], bf16)
        v_sb = const.tile([P, ST, HG, DK + 1], bf16)
        zT_sb = const.tile([P, MO, S], bf16)
        nc.gpsimd.memset(v_sb[:, :, :, DK:DK + 1], 1.0)
        # all-ones row used by the rank-1 reciprocal-broadcast matmul;
        # kept full-height so partition-64 slices exist.
        ones_sb = const.tile([P, DK], bf16)
        nc.gpsimd.memset(ones_sb[:], 1.0)
        warm_sb = const.tile([P, DK], bf16)
        nc.gpsimd.memset(warm_sb[:], 0.0)

        with (
            tc.tile_pool(name="pqkv", bufs=2, space="PSUM") as pqkv,
            tc.tile_pool(name="ps_at", bufs=2, space="PSUM") as ps_at,
            tc.tile_pool(name="pz", bufs=1, space="PSUM") as pz,
            tc.tile_pool(name="work", bufs=8) as work,
            tc.tile_pool(name="rwork", bufs=3) as rwork,
        ):
            # keep the PE busy during the DMA lead-in so the HAM clock
            # gate is already released when the first real matmul issues
            zp_warm = pz.tile([P, 2, NQ], f32, tag="z")
            for _ in range(28):
                nc.tensor.matmul(zp_warm[0:DK, 0, 0:DK], warm_sb[:, :],
                                 warm_sb[:, :], start=True, stop=True)

            def av(zp, mo, prev, last):
                kt, pT, off, first = prev
                ret = None
                for h in (0, 1):
                    ret = nc.tensor.matmul(
                        zp[0:DK + 1, h, off:],
                        v_sb[:, kt, 2 * mo + h, :],
                        pT[:, h, off:],
                        start=first, stop=last)
                return ret

            def outproj_pair(qt, pj, act_dma=False):
                # two adjacent output chunks share one SBUF tile and one
                # writeback DMA (DMA instruction issue is the scarce
                # resource); the two PSUM->SBUF copies go to different
                # engines so they drain in parallel.
                o_sb = work.tile([P, 2, NQ], bf16, tag="osb")
                for c in (0, 1):
                    mo8 = 2 * pj + c
                    o_ps = pqkv.tile([P, NQ], f32, tag="ps", name=f"o{mo8}")
                    for zo in range(MO):
                        nc.tensor.matmul(
                            o_ps, wo_sb[:, zo, ts(mo8, P)],
                            zT_sb[:, zo, ts(qt, NQ)],
                            start=(zo == 0), stop=(zo == MO - 1))
                    if c == 0:
                        nc.vector.tensor_copy(o_sb[:, c, :], o_ps)
                    else:
                        nc.scalar.copy(o_sb[:, c, :], o_ps)
                eng = nc.scalar if act_dma else nc.sync
                eng.dma_start(outv[:, 2 * pj:2 * pj + 2, ts(qt, NQ)],
                              o_sb[:])

            def qkv_slab(qt, mid_hook=None):
                for t, (w_sb, dst) in enumerate(((wq_sb, qT_sb),
                                                 (wk_sb, kT_sb))):
                    for mo in range(MO):
                        ps = pqkv.tile([P, NQ], f32, tag="ps")
                        for ko in range(KO):
                            nc.tensor.matmul(
                                ps, w_sb[:, ko, ts(mo, P)],
                                x_sb[:, ko, ts(qt, NQ)],
                                start=(ko == 0), stop=(ko == KO - 1))
                        nc.vector.tensor_scalar_add(
                            dst[:, mo, ts(qt, NQ)], ps,
                            bias_sb[:, t, mo:mo + 1])
                    if t == 0 and mid_hook is not None:
                        mid_hook()
                for so in range(HG * qt, HG * (qt + 1)):
                    ps = pqkv.tile([P, NQ], f32, tag="ps")
                    for ko in range(KO):
                        nc.tensor.matmul(
                            ps[:, :DG], x_sb[:, ko, ts(so, P)],
                            wv_sb[:, ko, :],
                            start=(ko == 0), stop=(ko == KO - 1))
                    nc.vector.tensor_copy(
                        v_sb[:, so, :, 0:DK],
                        ps[:, :DG].rearrange("p (h d) -> p h d", h=HG))

            # --- deferred softmax normalization ---------------------------
            # norm_a (emitted inline, right after the slab's last AV):
            #   reciprocal of the denominator row + copy of raw z out of
            #   PSUM (frees the bank).  norm_b (emitted a little later, so
            #   the PE queue has covering work): rank-1 broadcast matmul of
            #   the reciprocal + the normalizing multiplies.
            pending_nb = [None]

            def norm_a(mo, qt, zp, h1_on_act):
                # copy raw z (plus the denominator row DK) out of PSUM,
                # then a fast approximate reciprocal of that row.  The
                # copy frees the pz bank; exact reciprocal() costs
                # ~6.4ns/element and would stall the whole chain.
                zraw = rwork.tile([DK + 1, 2, NQ], f32, tag="zraw")
                if h1_on_act:
                    nc.vector.tensor_copy(zraw[:, 0, :], zp[:DK + 1, 0, :])
                    nc.scalar.copy(zraw[:, 1, :], zp[:DK + 1, 1, :])
                else:
                    nc.vector.tensor_copy(zraw[:], zp[:DK + 1, :, :])
                # reciprocal costs ~6.4ns/element of free size, so spread
                # the denominator row across all 128 partitions first
                # (small SBUF->SBUF DMAs), then gather back to one row for
                # the rank-1 broadcast matmul.
                # Issue the spread/gather DMAs from the SP queue, NOT the
                # ACT queue: a DMA waiting on its input blocks the issuing
                # engine's queue head, and ACT must keep streaming the
                # next head-pair's EXPs.
                NJ = 2 * NQ // P
                d_sp = rwork.tile([P, NJ], f32, tag="dsp")
                nc.sync.dma_start(d_sp[:], zraw[DK:DK + 1, :, :])
                r_sp = rwork.tile([P, NJ], bf16, tag="rsp")
                with nc.allow_low_precision("softmax denom tolerates bf16"):
                    nc.vector.reciprocal(r_sp[:], d_sp[:])
                r_row = rwork.tile([DK + 1, 2, NQ], bf16, tag="rrow")
                nc.sync.dma_start(r_row[DK:DK + 1, :, :], r_sp[:])
                pending_nb[0] = (mo, qt, zraw, r_row)

            def norm_b():
                if pending_nb[0] is None:
                    return
                mo, qt, zraw, r_row = pending_nb[0]
                pending_nb[0] = None
                # h1 first: its path is longer (partition-shift DMA after)
                for h in (1, 0):
                    rb = pqkv.tile([P, NQ], f32, tag="ps", name=f"rb{h}")
                    nc.tensor.matmul(
                        rb[0:DK, :], ones_sb[DK:DK + 1, :],
                        r_row[DK:DK + 1, h, :], start=True, stop=True)
                    if h == 0:
                        nc.vector.tensor_mul(
                            zT_sb[0:DK, mo, ts(qt, NQ)], zraw[0:DK, 0, :],
                            rb[0:DK, :])
                    else:
                        zn_tmp = rwork.tile([DK, NQ], bf16, tag="zt")
                        nc.vector.tensor_mul(zn_tmp[:], zraw[0:DK, 1, :],
                                             rb[0:DK, :])
                        nc.sync.dma_start(zT_sb[DK:P, mo, ts(qt, NQ)],
                                          zn_tmp[:])

            if not use_affine:
                # a general mask may attend beyond block qt, so all K/V
                # slabs must exist before any attention starts
                for qt in range(QT):
                    qkv_slab(qt)

            proc = list(range(QT))
            emitted = 0
            prev_qt = None
            for qt in proc:
                q0 = qt * NQ
                if use_affine:
                    # attention(qt) only needs k blocks <= qt, so emit QKV
                    # slabs lazily just ahead of it; the previous slab's
                    # last norm_b lands between the Q and K projections.
                    while emitted <= qt:
                        qkv_slab(emitted,
                                 mid_hook=norm_b if emitted == qt else None)
                        emitted += 1
                else:
                    norm_b()

                # -- attention over k blocks of this slab -----------------
                for mo in range(MO):
                    kts = [kt for kt in range(ST) if cls[kt][qt] != 0]
                    if not kts:
                        nc.vector.memset(zT_sb[:, mo, ts(qt, NQ)], 0.0)
                        continue
                    # out-projection of the previously processed q slab
                    # rides along inside this kt stream (mo==1); hold back
                    # pairs of the penultimate slab as covering PE work
                    # for the final normalization chain
                    outq = []
                    if mo == 1 and prev_qt is not None:
                        nch = 4 if qt != QT - 1 else 1
                        outq = [(prev_qt, j) for j in range(nch)]
                    per = -(-len(outq) // len(kts)) if outq else 0
                    zp = pz.tile([P, 2, NQ], f32, tag="z")
                    prev = None
                    for i, kt in enumerate(kts):
                        k0 = kt * P
                        off = qoff[kt][qt]
                        w = NQ - off
                        pT = work.tile([P, 2, NQ], bf16, tag="pT")
                        s_ps = ps_at.tile([P, 2, NQ], f32, tag="s")
                        for h in (0, 1):
                            hp = slice(h * DK, (h + 1) * DK)
                            nc.tensor.matmul(
                                s_ps[:, h, off:],
                                kT_sb[hp, mo, ts(kt, P)],
                                qT_sb[hp, mo, ds(q0 + off, w)],
                                start=True, stop=True,
                                tile_position=(h * DK, 0))
                        nc.scalar.activation(
                            pT[:, :, off:], s_ps[:, :, off:], Exp)
                        if cls[kt][qt] == 2:
                            if use_affine:
                                nc.gpsimd.affine_select(
                                    out=pT[:, :, off:],
                                    in_=pT[:, :, off:],
                                    compare_op=mybir.AluOpType.is_ge,
                                    fill=0.0,
                                    base=q0 + off - k0,
                                    channel_multiplier=-1,
                                    pattern=[[0, 2], [1, w]])
                            else:
                                nc.vector.tensor_mul(
                                    pT[:, :, off:], pT[:, :, off:],
                                    mask_sb[:, mixed_idx[(kt, qt)], None,
                                            off:].to_broadcast((P, 2, w)))
                        if prev is not None:
                            av(zp, mo, prev, last=False)
                        prev = (kt, pT, off, i == 0)
                        if mo == 1 and i == 2:
                            norm_b()   # this slab's mo=0 normalization
                        for qtd, j in outq[per * i: per * (i + 1)]:
                            outproj_pair(qtd, j)
                    for qtd, j in outq[per * len(kts):]:
                        outproj_pair(qtd, j)
                    av(zp, mo, prev, last=True)
                    norm_a(mo, qt, zp,
                           h1_on_act=(qt == proc[-1] and mo == MO - 1))
                prev_qt = qt

            # -- tail: held-back pairs cover the final norm chain ---------
            if QT >= 2:
                outproj_pair(QT - 2, 1)
                outproj_pair(QT - 2, 2)
                norm_b()
                outproj_pair(QT - 2, 3, act_dma=True)
            else:
                norm_b()
            for pj in range(D // P // 2):
                outproj_pair(prev_qt, pj, act_dma=(pj % 2 == 1))

    return nc


def _get_program(mask):
    cls, qoff, mixed_idx, mixed_tiles, use_affine = _classify_mask(mask)
    key = (use_affine,
           tuple(tuple(r) for r in cls),
           tuple(tuple(r) for r in qoff))
    if key not in _cache:
        nc = _build_program(cls, qoff, mixed_idx, len(mixed_tiles), use_affine)
        nc.compile()
        _cache[key] = nc
    return _cache[key], mixed_tiles


def _prep_in_maps(x, mask, Wq, bq, Wk, bk, Wv, bv, Wo, bo, mixed_tiles):
    xT = [np.ascontiguousarray(x[b].T).astype(BF16) for b in range(B)]
    in_maps = []
    for core in range(NCORES):
        b, g = divmod(core, GROUPS)
        c0, c1 = g * DG, (g + 1) * DG
        im = {
            "xT": xT[b],
            "wq": np.ascontiguousarray(Wq[:, c0:c1] * SCALE).astype(BF16),
            "wk": np.ascontiguousarray(Wk[:, c0:c1]).astype(BF16),
            "wv": np.ascontiguousarray(Wv[:, c0:c1]).astype(BF16),
            "wo": np.ascontiguousarray(Wo[c0:c1, :]).astype(BF16),
            "bqk": np.ascontiguousarray(
                np.stack([bq[c0:c1] * SCALE, bk[c0:c1]])).astype(np.float32),
        }
        if len(mixed_tiles):
            im["mmask"] = mixed_tiles
        in_maps.append(im)
    return in_maps


def _unshard(results, Wo, bv, bo):
    bo_eff = (bo.astype(np.float32)
              + bv.astype(np.float32) @ Wo.astype(np.float32))
    out = np.empty((B, S, D), np.float32)
    for b in range(B):
        acc = results[b * GROUPS]["outT"].astype(np.float32)
        for g in range(1, GROUPS):
            acc += results[b * GROUPS + g]["outT"]
        out[b] = acc.T + bo_eff
    return out


def kernel(trace=False, **inputs):
    from concourse import bass_utils

    args = {k: np.asarray(v) for k, v in inputs.items()}
    x, mask = args["x"], args["mask"]
    Wq, bq = args["Wq"], args["bq"]
    Wk, bk = args["Wk"], args["bk"]
    Wv, bv = args["Wv"], args["bv"]
    Wo, bo = args["Wo"], args["bo"]

    nc, mixed_tiles = _get_program(mask)
    in_maps = _prep_in_maps(x, mask, Wq, bq, Wk, bk, Wv, bv, Wo, bo,
                            mixed_tiles)
    res = bass_utils.run_bass_kernel_spmd(
        nc, in_maps, core_ids=list(range(NCORES)), trace=trace)
    out = _unshard(res.results, Wo, bv, bo)
    kernel.last_results = res
    return out


# revision 28
# speedup vs baseline: 1.3813x; 1.0043x over previous
"""Multi-head attention (B=2, S=2048, D=1024, H=16) on 8 Trainium2 cores.

Sharding: core = b*4 + g  ->  batch b (data parallel), head-group g of 4
heads (tensor parallel).  Each core computes a partial out^T = Wo_g^T @ Z_g
for its batch; the host sums the 4 partials per batch (the "all-reduce"),
transposes back and adds the (folded) output bias.

All activations flow feature-major on device (x^T, Q^T, K^T, scores^T) so
no on-device transposes are needed.  Matmuls run in bf16 with fp32 PSUM
accumulation.  Softmax skips the row-max pass (scores are bounded), gets
its denominator from a ones-column appended to V, and defers normalization
to after the attention*V matmul.  The reciprocal of the denominator is
broadcast across partitions with a rank-1 PE matmul (K=1) instead of a
DRAM round trip, and its instructions are emitted a few blocks late so
the in-order PE queue never stalls waiting on it.  Output partials are
written back in bf16 (the host accumulates in fp32).
"""

import numpy as np
import ml_dtypes

B, S, D, H = 2, 2048, 1024, 16
DK = D // H                  # 64
SCALE = 1.0 / np.sqrt(D)
NCORES = 8
GROUPS = 4                   # head-groups (tensor parallel)
HG = H // GROUPS             # 4 heads per group
DG = D // GROUPS             # 256 head dims per group
P = 128
KO = D // P                  # 8 contraction chunks for the projections
MO = DG // P                 # 2 row-chunks of Q^T/K^T (= head pairs)
NQ = 512                     # q tile width
QT = S // NQ                 # 4
ST = S // P                  # 16 key blocks / s chunks
BF16 = ml_dtypes.bfloat16

_cache = {}


def _classify_mask(mask):
    """Block structure of mask^T ([k, q] layout, P x NQ blocks).

    Returns (cls, qoff, mixed_idx, mixed_tiles, use_affine):
      cls[kt][qt]  : 0 all-masked, 1 all-kept, 2 mixed
      qoff[kt][qt] : leading all-masked columns (trim), 0 unless tril
      mixed_idx    : {(kt, qt): index into mixed_tiles}
      mixed_tiles  : np [n, P, NQ] bf16 0/1 tiles (empty when use_affine)
    """
    tril = np.tril(np.ones((S, S), dtype=mask.dtype))
    use_affine = bool(np.array_equal(mask, tril))
    cls = [[1] * QT for _ in range(ST)]
    qoff = [[0] * QT for _ in range(ST)]
    mixed_idx = {}
    tiles = []
    if use_affine:
        for kt in range(ST):
            k0 = kt * P
            for qt in range(QT):
                q0 = qt * NQ
                if k0 - q0 >= NQ:
                    cls[kt][qt] = 0
                elif k0 + P - 1 > q0:
                    cls[kt][qt] = 2
                    qoff[kt][qt] = min(max(k0 - q0, 0), NQ - P)
                # else: fully kept
    else:
        keepT = (mask != 0).T        # [k, q]
        for kt in range(ST):
            for qt in range(QT):
                blk = keepT[kt * P:(kt + 1) * P, qt * NQ:(qt + 1) * NQ]
                if not blk.any():
                    cls[kt][qt] = 0
                elif blk.all():
                    cls[kt][qt] = 1
                else:
                    cls[kt][qt] = 2
                    mixed_idx[(kt, qt)] = len(tiles)
                    tiles.append(blk.astype(BF16))
    mixed_tiles = (np.stack(tiles) if tiles else
                   np.zeros((0, P, NQ), dtype=BF16))
    return cls, qoff, mixed_idx, mixed_tiles, use_affine


def _build_program(cls, qoff, mixed_idx, n_mixed, use_affine):
    from contextlib import ExitStack
    import concourse.bass as bass
    import concourse.tile as tile
    import concourse.mybir as mybir
    from concourse import bacc
    from concourse.bass import ds, ts

    f32 = mybir.dt.float32
    bf16 = mybir.dt.bfloat16
    Exp = mybir.ActivationFunctionType.Exp

    nc = bacc.Bacc(None, target_bir_lowering=False, name="mha_tp")

    xT = nc.dram_tensor("xT", [D, S], bf16, kind="ExternalInput")
    wq = nc.dram_tensor("wq", [D, DG], bf16, kind="ExternalInput")
    wk = nc.dram_tensor("wk", [D, DG], bf16, kind="ExternalInput")
    wv = nc.dram_tensor("wv", [D, DG], bf16, kind="ExternalInput")
    wo = nc.dram_tensor("wo", [DG, D], bf16, kind="ExternalInput")
    bqk = nc.dram_tensor("bqk", [2, DG], f32, kind="ExternalInput")
    mm = (nc.dram_tensor("mmask", [n_mixed, P, NQ], bf16, kind="ExternalInput")
          if n_mixed else None)
    outT = nc.dram_tensor("outT", [D, S], bf16, kind="ExternalOutput")

    xTv = xT.ap().rearrange("(ko p) s -> p ko s", p=P)
    wqv = wq.ap().rearrange("(ko p) m -> p ko m", p=P)
    wkv = wk.ap().rearrange("(ko p) m -> p ko m", p=P)
    wvv = wv.ap().rearrange("(ko p) m -> p ko m", p=P)
    wov = wo.ap().rearrange("(zo p) n -> p zo n", p=P)
    bqkv = bqk.ap().rearrange("t (mo p) -> p t mo", p=P)
    outv = outT.ap().rearrange("(mo p) s -> p mo s", p=P)

    with tile.TileContext(nc) as tc, ExitStack() as ctx:
        const = ctx.enter_context(tc.tile_pool(name="const", bufs=1))

        # Each dma_start occupies its issuing sequencer for ~0.7us, so the
        # NUMBER of DMA instructions per queue matters as much as bytes.
        # SP carries wq + x (what the first matmuls need, finest pieces
        # first); ACT carries the rest of the weights during the lead-in
        # while it has nothing else to do.
        wq_sb = const.tile([P, KO, DG], bf16)
        x_sb = const.tile([P, KO, S], bf16)
        nc.sync.dma_start(wq_sb[:, :, ts(0, P)], wqv[:, :, ts(0, P)])
        for ko in range(0, KO, 2):
            nc.sync.dma_start(x_sb[:, ko:ko + 2, ts(0, NQ)],
                              xTv[:, ko:ko + 2, ts(0, NQ)])
            if ko == 2:
                nc.sync.dma_start(wq_sb[:, :, ts(1, P)], wqv[:, :, ts(1, P)])
        for qt in range(1, QT):
            nc.sync.dma_start(x_sb[:, :, ts(qt, NQ)], xTv[:, :, ts(qt, NQ)])
        bias_sb = const.tile([P, 2, 2], f32)
        nc.scalar.dma_start(bias_sb[:], bqkv)
        wk_sb = const.tile([P, KO, DG], bf16)
        nc.scalar.dma_start(wk_sb[:], wkv)
        wv_sb = const.tile([P, KO, DG], bf16)
        nc.scalar.dma_start(wv_sb[:], wvv)
        wo_sb = const.tile([P, MO, D], bf16)
        nc.scalar.dma_start(wo_sb[:], wov)
        mask_sb = None
        if n_mixed:
            mask_sb = const.tile([P, n_mixed, NQ], bf16)
            nc.scalar.dma_start(mask_sb[:], mm.ap())

        qT_sb = const.tile([P, MO, S], bf16)
        kT_sb = const.tile([P, MO, S], bf16)
        v_sb = const.tile([P, ST, HG, DK + 1], bf16)
        zT_sb = const.tile([P, MO, S], bf16)
        nc.gpsimd.memset(v_sb[:, :, :, DK:DK + 1], 1.0)
        # all-ones row used by the rank-1 reciprocal-broadcast matmul;
        # kept full-height so partition-64 slices exist.
        ones_sb = const.tile([P, DK], bf16)
        nc.gpsimd.memset(ones_sb[:], 1.0)
        warm_sb = const.tile([P, P], bf16)
        nc.gpsimd.memset(warm_sb[:], 0.0)

        with (
            tc.tile_pool(name="pqkv", bufs=2, space="PSUM") as pqkv,
            tc.tile_pool(name="ps_at", bufs=2, space="PSUM") as ps_at,
            tc.tile_pool(name="pz", bufs=1, space="PSUM") as pz,
            tc.tile_pool(name="work", bufs=8) as work,
            tc.tile_pool(name="rwork", bufs=3) as rwork,
        ):
            # keep the PE busy during the DMA lead-in so the HAM clock
            # gate is already released when the first real matmul issues
            zp_warm = pz.tile([P, 2, NQ], f32, tag="z")
            for _ in range(40):
                nc.tensor.matmul(zp_warm[:, 0, 0:P], warm_sb[:],
                                 warm_sb[:], start=True, stop=True)

            def av(zp, mo, prev, last):
                kt, pT, off, first = prev
                ret = None
                for h in (0, 1):
                    ret = nc.tensor.matmul(
                        zp[0:DK + 1, h, off:],
                        v_sb[:, kt, 2 * mo + h, :],
                        pT[:, h, off:],
                        start=first, stop=last)
                return ret

            def outproj_pair(qt, pj, act_dma=False):
                # two adjacent output chunks share one SBUF tile and one
                # writeback DMA (DMA instruction issue is the scarce
                # resource); the two PSUM->SBUF copies go to different
                # engines so they drain in parallel.
                o_sb = work.tile([P, 2, NQ], bf16, tag="osb")
                for c in (0, 1):
                    mo8 = 2 * pj + c
                    o_ps = pqkv.tile([P, NQ], f32, tag="ps", name=f"o{mo8}")
                    for zo in range(MO):
                        nc.tensor.matmul(
                            o_ps, wo_sb[:, zo, ts(mo8, P)],
                            zT_sb[:, zo, ts(qt, NQ)],
                            start=(zo == 0), stop=(zo == MO - 1))
                    if c == 0:
                        nc.vector.tensor_copy(o_sb[:, c, :], o_ps)
                    else:
                        nc.scalar.copy(o_sb[:, c, :], o_ps)
                eng = nc.scalar if act_dma else nc.sync
                eng.dma_start(outv[:, 2 * pj:2 * pj + 2, ts(qt, NQ)],
                              o_sb[:])

            def qkv_slab(qt, mid_hook=None):
                for t, (w_sb, dst) in enumerate(((wq_sb, qT_sb),
                                                 (wk_sb, kT_sb))):
                    for mo in range(MO):
                        ps = pqkv.tile([P, NQ], f32, tag="ps")
                        for ko in range(KO):
                            nc.tensor.matmul(
                                ps, w_sb[:, ko, ts(mo, P)],
                                x_sb[:, ko, ts(qt, NQ)],
                                start=(ko == 0), stop=(ko == KO - 1))
                        nc.vector.tensor_scalar_add(
                            dst[:, mo, ts(qt, NQ)], ps,
                            bias_sb[:, t, mo:mo + 1])
                    if t == 0 and mid_hook is not None:
                        mid_hook()
                for so in range(HG * qt, HG * (qt + 1)):
                    ps = pqkv.tile([P, NQ], f32, tag="ps")
                    for ko in range(KO):
                        nc.tensor.matmul(
                            ps[:, :DG], x_sb[:, ko, ts(so, P)],
                            wv_sb[:, ko, :],
                            start=(ko == 0), stop=(ko == KO - 1))
                    nc.vector.tensor_copy(
                        v_sb[:, so, :, 0:DK],
                        ps[:, :DG].rearrange("p (h d) -> p h d", h=HG))

            # --- deferred softmax normalization ---------------------------
            # norm_a (emitted inline, right after the slab's last AV):
            #   copy raw z out of PSUM (frees the bank), spread the
            #   denominator row across partitions, cheap reciprocal,
            #   gather back.  norm_b (emitted much later — at the NEXT
            #   slab's qkv boundary, when the chain's two DMA round trips
            #   have long completed): rank-1 broadcast matmul of the
            #   reciprocal + the normalizing multiplies.  zT(qt) is first
            #   consumed by outproj(qt) during slab qt+1, so this is safe.
            pending_nb = []

            def norm_a(mo, qt, zp, h1_on_act):
                # copy raw z (plus the denominator row DK) out of PSUM,
                # then a fast approximate reciprocal of that row.  The
                # copy frees the pz bank; exact reciprocal() costs
                # ~6.4ns/element and would stall the whole chain.
                zraw = rwork.tile([DK + 1, 2, NQ], f32, tag="zraw")
                if h1_on_act:
                    nc.vector.tensor_copy(zraw[:, 0, :], zp[:DK + 1, 0, :])
                    nc.scalar.copy(zraw[:, 1, :], zp[:DK + 1, 1, :])
                else:
                    nc.vector.tensor_copy(zraw[:], zp[:DK + 1, :, :])
                # reciprocal costs ~6.4ns/element of free size, so spread
                # the denominator row across all 128 partitions first
                # (small SBUF->SBUF DMAs), then gather back to one row for
                # the rank-1 broadcast matmul.
                # Issue the spread/gather DMAs from the SP queue, NOT the
                # ACT queue: a DMA waiting on its input blocks the issuing
                # engine's queue head, and ACT must keep streaming the
                # next head-pair's EXPs.
                NJ = 2 * NQ // P
                d_sp = rwork.tile([P, NJ], f32, tag="dsp")
                nc.sync.dma_start(d_sp[:], zraw[DK:DK + 1, :, :])
                r_sp = rwork.tile([P, NJ], bf16, tag="rsp")
                with nc.allow_low_precision("softmax denom tolerates bf16"):
                    nc.vector.reciprocal(r_sp[:], d_sp[:])
                r_row = rwork.tile([DK + 1, 2, NQ], bf16, tag="rrow")
                nc.sync.dma_start(r_row[DK:DK + 1, :, :], r_sp[:])
                pending_nb.append((mo, qt, zraw, r_row))

            def norm_b():
                if not pending_nb:
                    return
                mo, qt, zraw, r_row = pending_nb.pop(0)
                # h1 first: its path is longer (partition-shift DMA after)
                for h in (1, 0):
                    rb = pqkv.tile([P, NQ], f32, tag="ps", name=f"rb{h}")
                    nc.tensor.matmul(
                        rb[0:DK, :], ones_sb[DK:DK + 1, :],
                        r_row[DK:DK + 1, h, :], start=True, stop=True)
                    if h == 0:
                        nc.vector.tensor_mul(
                            zT_sb[0:DK, mo, ts(qt, NQ)], zraw[0:DK, 0, :],
                            rb[0:DK, :])
                    else:
                        zn_tmp = rwork.tile([DK, NQ], bf16, tag="zt")
                        nc.vector.tensor_mul(zn_tmp[:], zraw[0:DK, 1, :],
                                             rb[0:DK, :])
                        nc.sync.dma_start(zT_sb[DK:P, mo, ts(qt, NQ)],
                                          zn_tmp[:])

            if not use_affine:
                # a general mask may attend beyond block qt, so all K/V
                # slabs must exist before any attention starts
                for qt in range(QT):
                    qkv_slab(qt)

            proc = list(range(QT))
            emitted = 0
            prev_qt = None
            for qt in proc:
                q0 = qt * NQ
                if use_affine:
                    # attention(qt) only needs k blocks <= qt, so emit QKV
                    # slabs lazily just ahead of it; the previous slab's
                    # norm_bs land between the Q and K projections.
                    def flush():
                        norm_b()
                        norm_b()
                    while emitted <= qt:
                        qkv_slab(emitted,
                                 mid_hook=flush if emitted == qt else None)
                        emitted += 1
                else:
                    norm_b()
                    norm_b()

                # -- attention over k blocks of this slab -----------------
                for mo in range(MO):
                    kts = [kt for kt in range(ST) if cls[kt][qt] != 0]
                    if not kts:
                        nc.vector.memset(zT_sb[:, mo, ts(qt, NQ)], 0.0)
                        continue
                    # out-projection of the previously processed q slab
                    # rides along inside this kt stream (mo==1); hold back
                    # pairs of the penultimate slab as covering PE work
                    # for the final normalization chain
                    outq = []
                    if mo == 1 and prev_qt is not None:
                        nch = 4 if qt != QT - 1 else 1
                        outq = [(prev_qt, j) for j in range(nch)]
                    per = -(-len(outq) // len(kts)) if outq else 0
                    zp = pz.tile([P, 2, NQ], f32, tag="z")
                    prev = None
                    for i, kt in enumerate(kts):
                        k0 = kt * P
                        off = qoff[kt][qt]
                        w = NQ - off
                        pT = work.tile([P, 2, NQ], bf16, tag="pT")
                        s_ps = ps_at.tile([P, 2, NQ], f32, tag="s")
                        for h in (0, 1):
                            hp = slice(h * DK, (h + 1) * DK)
                            nc.tensor.matmul(
                                s_ps[:, h, off:],
                                kT_sb[hp, mo, ts(kt, P)],
                                qT_sb[hp, mo, ds(q0 + off, w)],
                                start=True, stop=True,
                                tile_position=(h * DK, 0))
                        nc.scalar.activation(
                            pT[:, :, off:], s_ps[:, :, off:], Exp)
                        if cls[kt][qt] == 2:
                            if use_affine:
                                nc.gpsimd.affine_select(
                                    out=pT[:, :, off:],
                                    in_=pT[:, :, off:],
                                    compare_op=mybir.AluOpType.is_ge,
                                    fill=0.0,
                                    base=q0 + off - k0,
                                    channel_multiplier=-1,
                                    pattern=[[0, 2], [1, w]])
                            else:
                                nc.vector.tensor_mul(
                                    pT[:, :, off:], pT[:, :, off:],
                                    mask_sb[:, mixed_idx[(kt, qt)], None,
                                            off:].to_broadcast((P, 2, w)))
                        if prev is not None:
                            av(zp, mo, prev, last=False)
                        prev = (kt, pT, off, i == 0)
                        for qtd, j in outq[per * i: per * (i + 1)]:
                            outproj_pair(qtd, j)
                    for qtd, j in outq[per * len(kts):]:
                        outproj_pair(qtd, j)
                    av(zp, mo, prev, last=True)
                    norm_a(mo, qt, zp,
                           h1_on_act=(qt == proc[-1] and mo == MO - 1))
                prev_qt = qt

            # -- tail: held-back pairs cover the final norm chain ---------
            norm_b()       # (mo=0, last qt) — its chain is long done
            if QT >= 2:
                outproj_pair(QT - 2, 1)
                outproj_pair(QT - 2, 2)
                outproj_pair(QT - 2, 3)
            norm_b()       # (mo=1, last qt)
            for pj in range(D // P // 2):
                outproj_pair(prev_qt, pj, act_dma=(pj % 2 == 1))

    return nc


def _get_program(mask):
    cls, qoff, mixed_idx, mixed_tiles, use_affine = _classify_mask(mask)
    key = (use_affine,
           tuple(tuple(r) for r in cls),
           tuple(tuple(r) for r in qoff))
    if key not in _cache:
        nc = _build_program(cls, qoff, mixed_idx, len(mixed_tiles), use_affine)
        nc.compile()
        _cache[key] = nc
    return _cache[key], mixed_tiles


def _prep_in_maps(x, mask, Wq, bq, Wk, bk, Wv, bv, Wo, bo, mixed_tiles):
    xT = [np.ascontiguousarray(x[b].T).astype(BF16) for b in range(B)]
    in_maps = []
    for core in range(NCORES):
        b, g = divmod(core, GROUPS)
        c0, c1 = g * DG, (g + 1) * DG
        im = {
            "xT": xT[b],
            "wq": np.ascontiguousarray(Wq[:, c0:c1] * SCALE).astype(BF16),
            "wk": np.ascontiguousarray(Wk[:, c0:c1]).astype(BF16),
            "wv": np.ascontiguousarray(Wv[:, c0:c1]).astype(BF16),
            "wo": np.ascontiguousarray(Wo[c0:c1, :]).astype(BF16),
            "bqk": np.ascontiguousarray(
                np.stack([bq[c0:c1] * SCALE, bk[c0:c1]])).astype(np.float32),
        }
        if len(mixed_tiles):
            im["mmask"] = mixed_tiles
        in_maps.append(im)
    return in_maps


def _unshard(results, Wo, bv, bo):
    bo_eff = (bo.astype(np.float32)
              + bv.astype(np.float32) @ Wo.astype(np.float32))
    out = np.empty((B, S, D), np.float32)
    for b in range(B):
        acc = results[b * GROUPS]["outT"].astype(np.float32)
        for g in range(1, GROUPS):
            acc += results[b * GROUPS + g]["outT"]
        out[b] = acc.T + bo_eff
    return out


def kernel(trace=False, **inputs):
    from concourse import bass_utils

    args = {k: np.asarray(v) for k, v in inputs.items()}
    x, mask = args["x"], args["mask"]
    Wq, bq = args["Wq"], args["bq"]
    Wk, bk = args["Wk"], args["bk"]
    Wv, bv = args["Wv"], args["bv"]
    Wo, bo = args["Wo"], args["bo"]

    nc, mixed_tiles = _get_program(mask)
    in_maps = _prep_in_maps(x, mask, Wq, bq, Wk, bk, Wv, bv, Wo, bo,
                            mixed_tiles)
    res = bass_utils.run_bass_kernel_spmd(
        nc, in_maps, core_ids=list(range(NCORES)), trace=trace)
    out = _unshard(res.results, Wo, bv, bo)
    kernel.last_results = res
    return out
